# revision 13
# baseline (speedup 1.0000x reference)
"""Trainium2 Bass kernel for the Mamba-style encoder problem.

Self-contained: builds and runs an 8-core SPMD Bass program, data-parallel
over the batch (2 sequences per core). Returns (m, v) like the reference.

Scan formulation: chunked (CH=127 steps + 1 carry row) cumulative-sum via
triangular matmul per state index n, exploiting d-uniform A (a_n scalar per
n).  The two local sequences' scans are interleaved chunk-by-chunk so their
independent dependency chains keep all engines busy.  Decay un-scaling uses
a vector divide by the same bf16 exp(+a_n P) tile used on the way in (errors
cancel); y accumulation runs on vector/gpsimd scalar_tensor_tensor instead
of diagonal matmuls.
"""
import os
import numpy as np
import ml_dtypes
from contextlib import ExitStack

import concourse.bass as bass
import concourse.bacc as bacc
import concourse.tile as tile
from concourse import mybir
from concourse.bass_utils import run_bass_kernel_spmd

F32 = mybir.dt.float32
BF16 = mybir.dt.bfloat16
AF = mybir.ActivationFunctionType
OP = mybir.AluOpType

B_LOC, L, EMB = 2, 2048, 512
NL, NS, DI, DC, DR = 4, 16, 1024, 4, 32
CH = 127                 # time steps per chunk
NCHUNK = (L + CH - 1) // CH   # 17 (last chunk 2048-16*127=16 steps)
DBLK = DI // 128         # 8
EBLK = EMB // 128        # 4


def bf(x):
    return np.ascontiguousarray(x).astype(ml_dtypes.bfloat16)


def host_prep(inputs, core_id):
    """Build the per-core in_map (numpy) from full inputs."""
    d = inputs
    b0 = core_id * B_LOC
    x = np.asarray(d["x"][b0:b0 + B_LOC])          # [2, 2048, 3]
    m = {}
    m["xT"] = np.ascontiguousarray(x.transpose(0, 2, 1)).astype(np.float32)  # [2,3,2048]
    m["fwT"] = np.asarray(d["t2v_freq_w"]).reshape(1, 511).astype(np.float32)
    fb = np.asarray(d["t2v_freq_b"]).astype(np.float32)
    fbb = np.zeros((128, 4), np.float32)
    for kb in range(4):
        fr0 = kb * 128; fr1 = min(511, (kb + 1) * 128)
        fbb[: fr1 - fr0, kb] = fb[fr0:fr1]
    m["fb"] = fbb
    iw = np.asarray(d["inp_w"])                      # [512, 515] cols: [s0,s1,delta,vlin,vper...]
    perm = list(range(4, 515)) + [3, 0, 1, 2]        # -> [vper(511), vlin, s0, s1, delta]
    m["inpwT"] = bf(iw[:, perm].T)                   # [515, 512]
    m["inpb"] = np.asarray(d["inp_b"]).reshape(4, 128).T.astype(np.float32).copy()
    m["lnw"] = np.ascontiguousarray(np.asarray(d["ln_w"]).reshape(NL, 4, 128).transpose(0, 2, 1)).astype(np.float32)
    m["lnb"] = np.ascontiguousarray(np.asarray(d["ln_b"]).reshape(NL, 4, 128).transpose(0, 2, 1)).astype(np.float32)
    m["ipwT"] = bf(np.asarray(d["in_proj_w"]).transpose(0, 2, 1))   # [4, 512, 2048]
    cw = np.asarray(d["conv_w"]).astype(np.float32)                  # [4, 1024, 4]
    m["convw"] = np.ascontiguousarray(cw.reshape(NL, 8, 128, 4).transpose(0, 2, 1, 3).reshape(NL, 128, 32))
    m["convb"] = np.ascontiguousarray(np.asarray(d["conv_b"]).reshape(NL, 8, 128).transpose(0, 2, 1)).astype(np.float32)
    xpt = np.asarray(d["x_proj_w"]).transpose(0, 2, 1)               # [4, 1024, 64]
    m["xpwT"] = bf(xpt.reshape(NL, 8, 128, 64).transpose(0, 2, 1, 3).reshape(NL, 128, 512))
    dtw = np.asarray(d["dt_w"])                                      # [4, 1024, 32]
    dtb = np.asarray(d["dt_b"])                                      # [4, 1024]
    dtwT_b = np.concatenate([dtw.transpose(0, 2, 1),
                             dtb[:, None, :]], axis=1)               # [4, 33, 1024]
    m["dtwT_b"] = dtwT_b.astype(np.float32)
    opt = np.asarray(d["out_proj_w"]).transpose(0, 2, 1)             # [4, 1024, 512]
    m["opwT"] = bf(opt.reshape(NL, 8, 128, 512).transpose(0, 2, 1, 3).reshape(NL, 128, 4096))
    m["Dp"] = np.ascontiguousarray(np.asarray(d["D"]).reshape(NL, 8, 128).transpose(0, 2, 1)).astype(np.float32)
    tri = np.tril(np.ones((128, 128), np.float32))
    m["triT_f32"] = np.ascontiguousarray(tri.T)      # lhsT for cumsum [s,t] upper-tri
    m["triT_bf"] = bf(tri.T)
    ident = np.eye(128, dtype=np.float32)
    m["ident_f32"] = ident
    identz = ident.copy(); identz[0, 0] = 0.0        # row-0-zeroed identity
    m["identz_bf"] = bf(identz)
    return m


def an_scales(inputs):
    """Per (layer, n) decay magnitudes a_n = -A[l,0,n]; assert d-uniform."""
    A_log = np.asarray(inputs["A_log"])  # [NL, DI, NS]
    A = -np.exp(A_log.astype(np.float64))
    spread = np.abs(A - A[:, :1, :]).max()
    assert spread < 1e-5 * max(1.0, np.abs(A).max()), \
        f"A_log not d-uniform (spread {spread}); kernel assumes per-n scalar decay"
    return (-A[:, 0, :]).astype(np.float64)   # [NL, NS] positive magnitudes


def declare_io(nc):
    io = {}
    def din(name, shape, dt):
        io[name] = nc.dram_tensor(name, list(shape), dt, kind="ExternalInput").ap()
    din("xT", (B_LOC, 3, L), F32)
    din("fwT", (1, 511), F32)
    din("fb", (128, 4), F32)
    din("inpwT", (515, 512), BF16)
    din("inpb", (128, 4), F32)
    din("lnw", (NL, 128, 4), F32)
    din("lnb", (NL, 128, 4), F32)
    din("ipwT", (NL, 512, 2048), BF16)
    din("convw", (NL, 128, 32), F32)
    din("convb", (NL, 128, 8), F32)
    din("xpwT", (NL, 128, 512), BF16)
    din("dtwT_b", (NL, 33, DI), F32)
    din("opwT", (NL, 128, 4096), BF16)
    din("Dp", (NL, 128, 8), F32)
    din("triT_f32", (128, 128), F32)
    din("triT_bf", (128, 128), BF16)
    din("ident_f32", (128, 128), F32)
    din("identz_bf", (128, 128), BF16)
    io["h_last"] = nc.dram_tensor("h_last", [B_LOC, EMB], F32, kind="ExternalOutput").ap()
    return io


def build_kernel(nc, io, an, w00, b0v):
    """Emit the full per-core program."""
    ctx = ExitStack()
    tc = ctx.enter_context(tile.TileContext(nc, pool_alloc_mode="queue"))

    consts = ctx.enter_context(tc.tile_pool(name="consts", bufs=1))
    wpool = ctx.enter_context(tc.tile_pool(name="wpool", bufs=1))
    xipool = ctx.enter_context(tc.tile_pool(name="xipool", bufs=1))
    dram = ctx.enter_context(tc.tile_pool(name="dram", bufs=1, space="DRAM"))
    psq = ctx.enter_context(tc.tile_pool(name="psq", bufs=2, space="PSUM"))
    psy = ctx.enter_context(tc.tile_pool(name="psy", bufs=1, space="PSUM"))

    h_dram = [dram.tile([EMB, L], F32, name=f"h_dram{b}", tag=f"h_dram{b}") for b in range(B_LOC)]
    xc_dram = [dram.tile([DI, L + 1], BF16, name=f"xc_dram{b}", tag=f"xc_dram{b}") for b in range(B_LOC)]
    sz_dram = [dram.tile([DI, L], BF16, name=f"sz_dram{b}", tag=f"sz_dram{b}") for b in range(B_LOC)]
    y_dram = [dram.tile([DI, L], BF16, name=f"y_dram{b}", tag=f"y_dram{b}") for b in range(B_LOC)]

    triT_bf = consts.tile([128, 128], BF16, name="triT_bf")
    nc.sync.dma_start(out=triT_bf, in_=io["triT_bf"])
    triT_f32 = consts.tile([128, 128], F32, name="triT_f32")
    nc.sync.dma_start(out=triT_f32, in_=io["triT_f32"])
    ident_f32 = consts.tile([128, 128], F32, name="ident_f32")
    nc.sync.dma_start(out=ident_f32, in_=io["ident_f32"])
    identz_bf = consts.tile([128, 128], BF16, name="identz_bf")
    nc.sync.dma_start(out=identz_bf, in_=io["identz_bf"])
    ones128 = consts.tile([128, 128], F32, name="ones128")
    nc.vector.memset(ones128, 1.0)
    eps_col = consts.tile([128, 1], F32, name="eps_col")
    nc.vector.memset(eps_col, 1e-5)
    b0_col = consts.tile([1, 1], F32, name="b0_col")
    nc.vector.memset(b0_col, float(b0v))
    fwT_sb = consts.tile([1, 511], F32, name="fwT_sb")
    nc.sync.dma_start(out=fwT_sb, in_=io["fwT"])
    fb_sb = consts.tile([128, 4], F32, name="fb_sb")
    nc.sync.dma_start(out=fb_sb, in_=io["fb"])

    # =====================================================================
    # Embedding -> h_dram[b]
    # =====================================================================
    with tc.tile_pool(name="epool", bufs=1) as epool:
        inpwT_sb = []
        for kb in range(5):
            k0, k1 = kb * 128, min(515, (kb + 1) * 128)
            t = epool.tile([k1 - k0, 512], BF16, name=f"inpwT{kb}", tag=f"inpwT{kb}")
            nc.sync.dma_start(out=t, in_=io["inpwT"][k0:k1, :])
            inpwT_sb.append(t)
        inpb_sb = epool.tile([128, 4], F32, name="inpb_sb", tag="inpb_sb")
        nc.sync.dma_start(out=inpb_sb, in_=io["inpb"])
        for b in range(B_LOC):
            trow = epool.tile([1, L + 1], F32, name="trow", tag="trow")
            nc.vector.memset(trow[:, 0:1], 0.0)
            nc.sync.dma_start(out=trow[:, 1:L + 1], in_=io["xT"][b, 2:3, :])
            featk = []
            for kb in range(5):
                kn = min(515, (kb + 1) * 128) - kb * 128
                featk.append(epool.tile([kn, L], BF16, name=f"feat{kb}", tag=f"feat{kb}"))
            # rows 0..510 = v_per ; 511 = v_lin ; 512,513 = s ; 514 = delta
            for kb in range(4):
                fr0, fr1 = kb * 128, min(511, (kb + 1) * 128)
                rn = fr1 - fr0
                for ts4 in range(4):
                    ps = psq.tile([rn, 512], F32, name="emb_ps", tag="q")
                    nc.tensor.matmul(ps, fwT_sb[:, fr0:fr1],
                                     trow[:, 1 + ts4 * 512:1 + (ts4 + 1) * 512],
                                     start=True, stop=True)
                    nc.scalar.activation(featk[kb][0:rn, ts4 * 512:(ts4 + 1) * 512],
                                         ps, AF.Sin, bias=fb_sb[0:rn, kb:kb + 1])
            stage_v = epool.tile([1, L], BF16, name="stage_v", tag="stage_v")
            nc.scalar.activation(stage_v, trow[:, 1:L + 1], AF.Identity,
                                 scale=float(w00), bias=b0_col)   # v_lin
            stage_d = epool.tile([1, L], BF16, name="stage_d", tag="stage_d")
            nc.vector.tensor_tensor(out=stage_d, in0=trow[:, 1:L + 1],
                                    in1=trow[:, 0:L], op=OP.subtract)  # delta
            nc.sync.dma_start(out=featk[3][127:128, :], in_=stage_v)
            nc.sync.dma_start(out=featk[4][2:3, :], in_=stage_d)
            s01 = epool.tile([2, L], F32, name="s01", tag="s01")
            nc.sync.dma_start(out=s01, in_=io["xT"][b, 0:2, :])
            nc.vector.tensor_copy(featk[4][0:2, :], s01)
            for eb in range(EBLK):
                hblk = epool.tile([128, L], F32, name="h0blk", tag="hblk0")
                for ts4 in range(4):
                    sl = slice(ts4 * 512, (ts4 + 1) * 512)
                    ps = psq.tile([128, 512], F32, name="h0ps", tag="q")
                    for kb in range(5):
                        nc.tensor.matmul(ps, inpwT_sb[kb][:, eb * 128:(eb + 1) * 128],
                                         featk[kb][:, sl], start=(kb == 0), stop=(kb == 4))
                    nc.scalar.activation(hblk[:, sl], ps, AF.Identity,
                                         bias=inpb_sb[:, eb:eb + 1])
                nc.sync.dma_start(out=h_dram[b][eb * 128:(eb + 1) * 128, :], in_=hblk)

    # =====================================================================
    # Layers
    # =====================================================================
    for l in range(NL):
        ipwT = []
        for kb in range(EBLK):
            t = wpool.tile([128, 2048], BF16, name=f"ipwT{kb}", tag=f"ipwT{kb}")
            nc.sync.dma_start(out=t, in_=io["ipwT"][l, kb * 128:(kb + 1) * 128, :])
            ipwT.append(t)
        opwT = wpool.tile([128, DBLK * 512], BF16, name="opwT", tag="opwT")
        nc.sync.dma_start(out=opwT, in_=io["opwT"][l])
        xpwT = wpool.tile([128, DBLK * 64], BF16, name="xpwT", tag="xpwT")
        nc.sync.dma_start(out=xpwT, in_=io["xpwT"][l])
        dtwT = wpool.tile([33, DI], F32, name="dtwT", tag="dtwT")
        nc.sync.dma_start(out=dtwT, in_=io["dtwT_b"][l, :, :])
        lnwb = wpool.tile([128, 8], F32, name="lnwb", tag="lnwb")   # cols 0-3 w, 4-7 b
        nc.sync.dma_start(out=lnwb[:, 0:4], in_=io["lnw"][l])
        nc.sync.dma_start(out=lnwb[:, 4:8], in_=io["lnb"][l])
        convw = wpool.tile([128, 32], F32, name="convw", tag="convw")
        nc.sync.dma_start(out=convw, in_=io["convw"][l])
        cbd = wpool.tile([128, 16], F32, name="cbd", tag="cbd")     # cols 0-7 convb, 8-15 D
        nc.sync.dma_start(out=cbd[:, 0:8], in_=io["convb"][l])
        nc.sync.dma_start(out=cbd[:, 8:16], in_=io["Dp"][l])

        with tc.tile_pool(name="midp", bufs=1) as midp:
            dtr_pad = []
            bc_pad = []
            for b in range(B_LOC):
                dp = midp.tile([33, L + 1], F32, name=f"dtr_pad{b}", tag=f"dtr_pad{b}")
                bp = midp.tile([32, L + 1], F32, name=f"bc_pad{b}", tag=f"bc_pad{b}")
                nc.vector.memset(dp[32:33, :], 1.0)
                nc.vector.memset(dp[0:32, 0:1], 0.0)   # col 0 never written below
                nc.vector.memset(bp[:, 0:1], 0.0)      # col 0 garbage -> NaN via 0*inf
                dtr_pad.append(dp)
                bc_pad.append(bp)
            xc_sbuf = {b: [] for b in range(B_LOC)}

            # ============ phase A (per sequence) ============
            for b in range(B_LOC):
                with tc.tile_pool(name="apool", bufs=1) as apool, \
                     tc.tile_pool(name="hstr", bufs=2) as hstr:
                    mu_bc = apool.tile([128, L], F32, name="mu_bc", tag="mu_bc")
                    rstd_bc = apool.tile([128, L], F32, name="rstd_bc", tag="rstd_bc")
                    pm = [psq.tile([128, 1024], F32, name=f"pm{i}", tag="q") for i in range(2)]
                    for eb in range(EBLK):
                        hb = hstr.tile([128, L], F32, name="h_in", tag="hblk")
                        nc.sync.dma_start(out=hb, in_=h_dram[b][eb * 128:(eb + 1) * 128, :])
                        for h2 in range(2):
                            for q2 in range(2):
                                nc.tensor.matmul(pm[h2][:, q2 * 512:(q2 + 1) * 512], ones128,
                                                 hb[:, h2 * 1024 + q2 * 512:h2 * 1024 + (q2 + 1) * 512],
                                                 start=(eb == 0), stop=(eb == EBLK - 1))
                    for h2 in range(2):
                        nc.scalar.activation(mu_bc[:, h2 * 1024:(h2 + 1) * 1024], pm[h2],
                                             AF.Copy, scale=1.0 / EMB)
                    pm2 = [psq.tile([128, 1024], F32, name=f"pm2{i}", tag="q") for i in range(2)]
                    for eb in range(EBLK):
                        hb = hstr.tile([128, L], F32, name="h_in2", tag="hblk")
                        nc.sync.dma_start(out=hb, in_=h_dram[b][eb * 128:(eb + 1) * 128, :])
                        sqs = apool.tile([128, L], F32, name="sqs", tag="scratch8k")
                        nc.vector.tensor_tensor(out=sqs, in0=hb, in1=hb, op=OP.mult)
                        for h2 in range(2):
                            for q2 in range(2):
                                nc.tensor.matmul(pm2[h2][:, q2 * 512:(q2 + 1) * 512], ones128,
                                                 sqs[:, h2 * 1024 + q2 * 512:h2 * 1024 + (q2 + 1) * 512],
                                                 start=(eb == 0), stop=(eb == EBLK - 1))
                    mu2 = apool.tile([128, L], F32, name="mu2", tag="scratch8k")
                    nc.vector.tensor_tensor(out=mu2, in0=mu_bc, in1=mu_bc, op=OP.mult)
                    for h2 in range(2):
                        sl2 = slice(h2 * 1024, (h2 + 1) * 1024)
                        nc.vector.scalar_tensor_tensor(out=rstd_bc[:, sl2], in0=pm2[h2],
                                                       scalar=1.0 / EMB, in1=mu2[:, sl2],
                                                       op0=OP.mult, op1=OP.subtract)
                    nc.scalar.activation(rstd_bc, rstd_bc, AF.Ln, bias=eps_col)
                    nc.scalar.activation(rstd_bc, rstd_bc, AF.Exp, scale=-0.5)
                    hn = []
                    for eb in range(EBLK):
                        hb = hstr.tile([128, L], F32, name="h_in3", tag="hblk")
                        nc.sync.dma_start(out=hb, in_=h_dram[b][eb * 128:(eb + 1) * 128, :])
                        t1 = apool.tile([128, L], F32, name="lnt1", tag="scratch8k")
                        nc.vector.tensor_tensor(out=t1, in0=hb, in1=mu_bc, op=OP.subtract)
                        nc.vector.tensor_tensor(out=t1, in0=t1, in1=rstd_bc, op=OP.mult)
                        hnb = apool.tile([128, L], BF16, name=f"hn{eb}", tag=f"hn{eb}")
                        nc.scalar.activation(hnb, t1, AF.Identity,
                                             scale=lnwb[:, eb:eb + 1], bias=lnwb[:, 4 + eb:5 + eb])
                        hn.append(hnb)
                    # ---- in_proj ----
                    xi_blocks = []
                    for ob in range(16):
                        is_x = ob < 8
                        if is_x:
                            dst = xipool.tile([128, L + DC - 1], BF16, name=f"xi{ob}",
                                              tag=f"xi{ob % 8}")
                            nc.vector.memset(dst[:, 0:DC - 1], 0.0)
                            xi_blocks.append(dst)
                        else:
                            dst = apool.tile([128, L], BF16, name="zblk", tag="zblk")
                        for ts4 in range(4):
                            sl = slice(ts4 * 512, (ts4 + 1) * 512)
                            ps = psq.tile([128, 512], F32, name="ip_ps", tag="q")
                            for kb in range(EBLK):
                                nc.tensor.matmul(ps, ipwT[kb][:, ob * 128:(ob + 1) * 128],
                                                 hn[kb][:, sl], start=(kb == 0), stop=(kb == EBLK - 1))
                            if is_x:
                                nc.scalar.activation(dst[:, DC - 1 + ts4 * 512:DC - 1 + (ts4 + 1) * 512],
                                                     ps, AF.Copy)
                            else:
                                nc.scalar.activation(dst[:, sl], ps, AF.Silu)
                        if not is_x:
                            nc.sync.dma_start(out=sz_dram[b][(ob - 8) * 128:(ob - 7) * 128, :], in_=dst)
                    # ---- conv ----
                    for db in range(DBLK):
                        xi = xi_blocks[db]
                        t_a = apool.tile([128, L], F32, name="t_a", tag="scratch8k")
                        nc.vector.tensor_scalar_mul(t_a, xi[:, 0:L], convw[:, db * 4:db * 4 + 1])
                        for k in range(1, DC):
                            nc.vector.scalar_tensor_tensor(
                                out=t_a, in0=xi[:, k:k + L],
                                scalar=convw[:, db * 4 + k:db * 4 + k + 1],
                                in1=t_a, op0=OP.mult, op1=OP.add)
                        xcb = xipool.tile([128, L], BF16, name=f"xc{db}", tag=f"xi{db}")
                        nc.scalar.activation(xcb, t_a, AF.Silu, bias=cbd[:, db:db + 1])
                        nc.sync.dma_start(out=xc_dram[b][db * 128:(db + 1) * 128, 1:L + 1], in_=xcb)
                        xc_sbuf[b].append(xcb)
                    # ---- x_proj ----
                    for ts4 in range(4):
                        sl = slice(ts4 * 512, (ts4 + 1) * 512)
                        slp = slice(1 + ts4 * 512, 1 + (ts4 + 1) * 512)
                        ps = psq.tile([64, 512], F32, name="xp_ps", tag="q")
                        for kb in range(DBLK):
                            nc.tensor.matmul(ps, xpwT[:, kb * 64:(kb + 1) * 64],
                                             xc_sbuf[b][kb][:, sl],
                                             start=(kb == 0), stop=(kb == DBLK - 1))
                        nc.scalar.activation(dtr_pad[b][0:32, slp], ps[0:32, :], AF.Copy)
                        nc.scalar.activation(bc_pad[b][:, slp], ps[32:64, :], AF.Copy)

            # ============ scan: both sequences interleaved chunk-by-chunk ====
            with tc.tile_pool(name="sp1", bufs=1) as sp1, \
                 tc.tile_pool(name="sp2", bufs=2) as sp2, \
                 tc.tile_pool(name="gpool", bufs=1) as gpool:
                carry_sb = []
                for b in range(B_LOC):
                    cs = sp1.tile([NS, DI], BF16, name=f"carry{b}", tag=f"carry{b}")
                    nc.vector.memset(cs, 0.0)
                    carry_sb.append(cs)
                for c in range(NCHUNK):
                    steps = min(CH, L - c * CH)
                    rows = steps + 1
                    full = (steps == CH)
                    for b in range(B_LOC):
                        ps_dt = psq.tile([rows, DI], F32, name="ps_dt", tag="q")
                        lhs_dtr = dtr_pad[b][:, c * CH:c * CH + rows]
                        for h2 in range(2):
                            nc.tensor.matmul(ps_dt[:, h2 * 512:(h2 + 1) * 512],
                                             lhs_dtr, dtwT[:, h2 * 512:(h2 + 1) * 512],
                                             start=True, stop=True)
                        dt_t = sp1.tile([128, DI], F32, name="dt_t", tag=f"dt_t{b}")
                        if not full:
                            nc.vector.memset(dt_t, 0.0)
                        nc.scalar.activation(dt_t[0:rows, :], ps_dt, AF.Exp)
                        nc.scalar.activation(dt_t[0:rows, :], dt_t[0:rows, :], AF.Ln, bias=1.0)
                        nc.gpsimd.memset(dt_t[0:1, :], 0.0)
                        ps_P = psq.tile([128, DI], F32, name="ps_P", tag="q")
                        for h2 in range(2):
                            nc.tensor.matmul(ps_P[:, h2 * 512:(h2 + 1) * 512],
                                             triT_f32, dt_t[:, h2 * 512:(h2 + 1) * 512],
                                             start=True, stop=True)
                        P_sb = sp1.tile([128, DI], F32, name="P_sb", tag=f"P_sb{b}")
                        nc.scalar.activation(P_sb, ps_P, AF.Copy)
                        u_t = sp1.tile([128, DI], BF16, name="u_t", tag=f"u_t{b}")
                        nc.sync.dma_start_transpose(u_t[0:rows, :],
                                                    xc_dram[b][:, c * CH:c * CH + rows])
                        if c == 0:
                            nc.vector.memset(u_t[0:1, :], 0.0)  # xc col 0 is uninit DRAM
                        nc.vector.tensor_tensor(out=u_t[0:rows, :], in0=dt_t[0:rows, :],
                                                in1=u_t[0:rows, :], op=OP.mult)
                        ps_bc = psq.tile([rows, 32], F32, name="ps_bc", tag="q")
                        nc.tensor.transpose(ps_bc, bc_pad[b][:, c * CH:c * CH + rows],
                                            ident_f32[0:32, 0:32])
                        bc_cols = sp1.tile([128, 32], F32, name="bc_cols", tag=f"bc_cols{b}")
                        nc.scalar.activation(bc_cols[0:rows, :], ps_bc, AF.Copy)
                        ps_y = psy.tile([128, DI], F32, name="ps_y", tag=f"psy{b}")
                        for g in range(4):
                            Gg = gpool.tile([128, 4 * DI], BF16, name="Gg", tag=f"Gg{b}")
                            e1g = gpool.tile([128, 4 * DI], BF16, name="e1g", tag=f"epg{b}")
                            if not full:
                                nc.vector.memset(Gg, 0.0)
                                nc.vector.memset(e1g, 0.0)
                            for j in range(4):
                                n = g * 4 + j
                                a_n = float(an[l, n])
                                dsl = slice(j * DI, (j + 1) * DI)
                                E1p = sp2.tile([128, DI], BF16, name="E1p", tag="E1p")
                                nc.scalar.activation(E1p[0:rows, :], P_sb[0:rows, :],
                                                     AF.Exp, scale=a_n)
                                if n % 2 == 0:
                                    # exact elementwise reciprocal cancels with E1p
                                    with nc.allow_low_precision(reason="bf16 decay factors"):
                                        nc.vector.reciprocal(out=e1g[0:rows, dsl],
                                                             in_=E1p[0:rows, :])
                                else:
                                    nc.scalar.activation(e1g[0:rows, dsl], P_sb[0:rows, :],
                                                         AF.Exp, scale=-a_n)
                                bu = sp2.tile([128, DI], BF16, name="bu", tag="bu")
                                nc.vector.tensor_scalar_mul(bu[0:rows, :], u_t[0:rows, :],
                                                            bc_cols[0:rows, n:n + 1])
                                nc.gpsimd.tensor_tensor(out=Gg[0:rows, dsl],
                                                        in0=E1p[0:rows, :],
                                                        in1=bu[0:rows, :], op=OP.mult)
                            nc.gpsimd.dma_start(out=Gg[0:1, :],
                                                in_=carry_sb[b][g * 4:(g + 1) * 4, :])
                            for j in range(4):
                                n = g * 4 + j
                                dsl = slice(j * DI, (j + 1) * DI)
                                ps_q = psq.tile([128, DI], F32, name="ps_q", tag="q")
                                for h2 in range(2):
                                    nc.tensor.matmul(ps_q[:, h2 * 512:(h2 + 1) * 512], triT_bf,
                                                     Gg[:, j * DI + h2 * 512:j * DI + (h2 + 1) * 512],
                                                     start=True, stop=True)
                                nc.vector.tensor_tensor(out=Gg[:, dsl], in0=ps_q,
                                                        in1=e1g[:, dsl], op=OP.mult)
                                diag = sp2.tile([128, 128], BF16, name="diag", tag="diag")
                                nc.gpsimd.tensor_scalar_mul(diag, identz_bf,
                                                            bc_cols[:, 16 + n:17 + n])
                                for h2 in range(2):
                                    nc.tensor.matmul(ps_y[:, h2 * 512:(h2 + 1) * 512], diag,
                                                     Gg[:, j * DI + h2 * 512:j * DI + (h2 + 1) * 512],
                                                     start=(n == 0), stop=(n == NS - 1),
                                                     skip_group_check=True)
                            if c < NCHUNK - 1:
                                nc.gpsimd.dma_start(out=carry_sb[b][g * 4:(g + 1) * 4, :],
                                                    in_=Gg[CH:CH + 1, :])
                        y_sb = sp1.tile([128, DI], F32, name="y_sb", tag=f"y_sb{b}")
                        nc.scalar.activation(y_sb, ps_y, AF.Copy)
                        for db in range(DBLK):
                            ps_t = psq.tile([128, 128], F32, name="ps_t", tag="q")
                            nc.tensor.transpose(ps_t, y_sb[:, db * 128:(db + 1) * 128], ident_f32)
                            ytile = sp2.tile([128, CH], BF16, name="ytile", tag="ytile")
                            nc.scalar.activation(ytile[:, 0:steps], ps_t[:, 1:rows], AF.Copy)
                            nc.scalar.dma_start(
                                out=y_dram[b][db * 128:(db + 1) * 128, c * CH:c * CH + steps],
                                in_=ytile[:, 0:steps])

            # ============ epilogue (per sequence, in L-halves) ============
            for b in range(B_LOC):
                with tc.tile_pool(name="epi", bufs=1) as epi:
                    LH = L // 2
                    for lh in range(2):
                        c0 = lh * LH
                        y2b = []
                        for db in range(DBLK):
                            yb = epi.tile([128, LH], BF16, name="yb", tag=f"yb{db % 2}")
                            nc.sync.dma_start(out=yb,
                                              in_=y_dram[b][db * 128:(db + 1) * 128, c0:c0 + LH])
                            szb = epi.tile([128, LH], BF16, name="szb", tag=f"szb{db % 2}")
                            nc.sync.dma_start(out=szb,
                                              in_=sz_dram[b][db * 128:(db + 1) * 128, c0:c0 + LH])
                            xcb_e = epi.tile([128, LH], BF16, name="xcb_e", tag=f"xcb_e{db % 2}")
                            nc.sync.dma_start(out=xcb_e,
                                              in_=xc_dram[b][db * 128:(db + 1) * 128,
                                                             1 + c0:1 + c0 + LH])
                            y2 = epi.tile([128, LH], BF16, name=f"y2_{db}", tag=f"y2_{db}")
                            nc.vector.scalar_tensor_tensor(out=y2, in0=xcb_e,
                                                           scalar=cbd[:, 8 + db:9 + db],
                                                           in1=yb, op0=OP.mult, op1=OP.add)
                            nc.vector.tensor_tensor(out=y2, in0=y2, in1=szb, op=OP.mult)
                            y2b.append(y2)
                        for eb in range(EBLK):
                            hb2 = epi.tile([128, LH], F32, name="h_out", tag=f"h_out{eb % 2}")
                            nc.sync.dma_start(out=hb2,
                                              in_=h_dram[b][eb * 128:(eb + 1) * 128, c0:c0 + LH])
                            for ts2 in range(2):
                                sl = slice(ts2 * 512, (ts2 + 1) * 512)
                                ps = psq.tile([128, 512], F32, name="op_ps", tag="q")
                                for kb in range(DBLK):
                                    nc.tensor.matmul(ps, opwT[:, kb * 512 + eb * 128:kb * 512 + (eb + 1) * 128],
                                                     y2b[kb][:, sl], start=(kb == 0), stop=(kb == DBLK - 1))
                                nc.vector.tensor_tensor(out=hb2[:, sl], in0=hb2[:, sl],
                                                        in1=ps, op=OP.add)
                            nc.sync.dma_start(out=h_dram[b][eb * 128:(eb + 1) * 128, c0:c0 + LH],
                                              in_=hb2)

    for b in range(B_LOC):
        nc.sync.dma_start(out=io["h_last"][b:b + 1, :],
                          in_=h_dram[b][:, L - 1:L].rearrange("e o -> o e"))

    ctx.close()
    return {}


_CACHE = {}


def _install_ntff_shim():
    import sys, types
    if "antenv.axon_hooks" in sys.modules:
        return
    try:
        mod = types.ModuleType("antenv.axon_hooks")
        mod._hook = None
        def set_axon_ntff_profile_hook(h): mod._hook = h
        def get_axon_ntff_profile_hook(): return mod._hook
        mod.set_axon_ntff_profile_hook = set_axon_ntff_profile_hook
        mod.get_axon_ntff_profile_hook = get_axon_ntff_profile_hook
        import antenv
        antenv.axon_hooks = mod
        sys.modules["antenv.axon_hooks"] = mod
        from trn_agent_boot.trn_boot import _ntff_profile_via_ctypes
        hook = _ntff_profile_via_ctypes("/opt/axon/libaxon_pjrt.so")
        set_axon_ntff_profile_hook(hook)
    except Exception:
        pass


def _get_program(an, w00, b0v):
    key = (tuple(np.asarray(an).ravel().tolist()), float(w00), float(b0v))
    if key in _CACHE:
        return _CACHE[key]
    nc = bacc.Bacc("TRN2", target_bir_lowering=False, debug=False, num_devices=8)
    io = declare_io(nc)
    build_kernel(nc, io, an, w00, b0v)
    nc.compile()
    _CACHE[key] = nc
    return nc


def _softplus(x):
    return np.log1p(np.exp(-np.abs(x))) + np.maximum(x, 0)


def kernel(**inputs):
    trace = bool(int(os.environ.get("BASS_KERNEL_TRACE", "0")))
    if trace:
        _install_ntff_shim()
    an = an_scales(inputs)
    w00 = float(np.asarray(inputs["t2v_lin_w"])[0, 0])
    b0v = float(np.asarray(inputs["t2v_lin_b"])[0])
    nc = _get_program(an, w00, b0v)
    in_maps = [host_prep(inputs, c) for c in range(8)]
    res = run_bass_kernel_spmd(nc, in_maps, core_ids=list(range(8)), trace=trace)
    if trace and res.exec_time_ns is not None:
        print(f"HW exec time: {res.exec_time_ns} ns")
        kernel.last_exec_time_ns = res.exec_time_ns
    h_last = np.concatenate([r["h_last"] for r in res.results], axis=0)  # [16, 512]
    dec_w = np.asarray(inputs["dec_w"], np.float32)
    dec_b = np.asarray(inputs["dec_b"], np.float32)
    stats = h_last.astype(np.float32) @ dec_w.T + dec_b
    m, v_ = np.split(stats, 2, axis=-1)
    v = _softplus(v_) + 1e-5
    return (m.astype(np.float32), v.astype(np.float32))


# revision 15
# speedup vs baseline: 1.3882x; 1.3882x over previous
"""Trainium2 Bass kernel for the Mamba-style encoder problem.

Self-contained: builds and runs an 8-core SPMD Bass program, data-parallel
over the batch (2 sequences per core). Returns (m, v) like the reference.

Scan formulation: chunked (CH=127 steps + 1 carry row) cumulative-sum via
triangular matmul per state index n, exploiting d-uniform A (a_n scalar per
n).  The two local sequences' scans are interleaved chunk-by-chunk so their
independent dependency chains keep all engines busy.  Decay un-scaling uses
a vector divide by the same bf16 exp(+a_n P) tile used on the way in (errors
cancel); y accumulation runs on vector/gpsimd scalar_tensor_tensor instead
of diagonal matmuls.
"""
import os
import numpy as np
import ml_dtypes
from contextlib import ExitStack

import concourse.bass as bass
import concourse.bacc as bacc
import concourse.tile as tile
from concourse import mybir
from concourse.bass_utils import run_bass_kernel_spmd

F32 = mybir.dt.float32
BF16 = mybir.dt.bfloat16
AF = mybir.ActivationFunctionType
OP = mybir.AluOpType

B_LOC, L, EMB = 2, 2048, 512
NL, NS, DI, DC, DR = 4, 16, 1024, 4, 32
CH = 127                 # time steps per chunk
NCHUNK = (L + CH - 1) // CH   # 17 (last chunk 2048-16*127=16 steps)
DBLK = DI // 128         # 8
EBLK = EMB // 128        # 4


def bf(x):
    return np.ascontiguousarray(x).astype(ml_dtypes.bfloat16)


def host_prep(inputs, core_id):
    """Build the per-core in_map (numpy) from full inputs."""
    d = inputs
    b0 = core_id * B_LOC
    x = np.asarray(d["x"][b0:b0 + B_LOC])          # [2, 2048, 3]
    m = {}
    m["xT"] = np.ascontiguousarray(x.transpose(0, 2, 1)).astype(np.float32)  # [2,3,2048]
    m["fwT"] = np.asarray(d["t2v_freq_w"]).reshape(1, 511).astype(np.float32)
    fb = np.asarray(d["t2v_freq_b"]).astype(np.float32)
    fbb = np.zeros((128, 4), np.float32)
    for kb in range(4):
        fr0 = kb * 128; fr1 = min(511, (kb + 1) * 128)
        fbb[: fr1 - fr0, kb] = fb[fr0:fr1]
    m["fb"] = fbb
    iw = np.asarray(d["inp_w"])                      # [512, 515] cols: [s0,s1,delta,vlin,vper...]
    perm = list(range(4, 515)) + [3, 0, 1, 2]        # -> [vper(511), vlin, s0, s1, delta]
    m["inpwT"] = bf(iw[:, perm].T)                   # [515, 512]
    m["inpb"] = np.asarray(d["inp_b"]).reshape(4, 128).T.astype(np.float32).copy()
    m["lnw"] = np.ascontiguousarray(np.asarray(d["ln_w"]).reshape(NL, 4, 128).transpose(0, 2, 1)).astype(np.float32)
    m["lnb"] = np.ascontiguousarray(np.asarray(d["ln_b"]).reshape(NL, 4, 128).transpose(0, 2, 1)).astype(np.float32)
    m["ipwT"] = bf(np.asarray(d["in_proj_w"]).transpose(0, 2, 1))   # [4, 512, 2048]
    cw = np.asarray(d["conv_w"]).astype(np.float32)                  # [4, 1024, 4]
    m["convw"] = np.ascontiguousarray(cw.reshape(NL, 8, 128, 4).transpose(0, 2, 1, 3).reshape(NL, 128, 32))
    m["convb"] = np.ascontiguousarray(np.asarray(d["conv_b"]).reshape(NL, 8, 128).transpose(0, 2, 1)).astype(np.float32)
    xpt = np.asarray(d["x_proj_w"]).transpose(0, 2, 1)               # [4, 1024, 64]
    m["xpwT"] = bf(xpt.reshape(NL, 8, 128, 64).transpose(0, 2, 1, 3).reshape(NL, 128, 512))
    dtw = np.asarray(d["dt_w"])                                      # [4, 1024, 32]
    dtb = np.asarray(d["dt_b"])                                      # [4, 1024]
    dtwT_b = np.concatenate([dtw.transpose(0, 2, 1),
                             dtb[:, None, :]], axis=1)               # [4, 33, 1024]
    m["dtwT_b"] = dtwT_b.astype(np.float32)
    opt = np.asarray(d["out_proj_w"]).transpose(0, 2, 1)             # [4, 1024, 512]
    m["opwT"] = bf(opt.reshape(NL, 8, 128, 512).transpose(0, 2, 1, 3).reshape(NL, 128, 4096))
    m["Dp"] = np.ascontiguousarray(np.asarray(d["D"]).reshape(NL, 8, 128).transpose(0, 2, 1)).astype(np.float32)
    tri = np.tril(np.ones((128, 128), np.float32))
    m["triT_f32"] = np.ascontiguousarray(tri.T)      # lhsT for cumsum [s,t] upper-tri
    m["triT_bf"] = bf(tri.T)
    ident = np.eye(128, dtype=np.float32)
    m["ident_f32"] = ident
    identz = ident.copy(); identz[0, 0] = 0.0        # row-0-zeroed identity
    m["identz_bf"] = bf(identz)
    return m


def an_scales(inputs):
    """Per (layer, n) decay magnitudes a_n = -A[l,0,n]; assert d-uniform."""
    A_log = np.asarray(inputs["A_log"])  # [NL, DI, NS]
    A = -np.exp(A_log.astype(np.float64))
    spread = np.abs(A - A[:, :1, :]).max()
    assert spread < 1e-5 * max(1.0, np.abs(A).max()), \
        f"A_log not d-uniform (spread {spread}); kernel assumes per-n scalar decay"
    return (-A[:, 0, :]).astype(np.float64)   # [NL, NS] positive magnitudes


def declare_io(nc):
    io = {}
    def din(name, shape, dt):
        io[name] = nc.dram_tensor(name, list(shape), dt, kind="ExternalInput").ap()
    din("xT", (B_LOC, 3, L), F32)
    din("fwT", (1, 511), F32)
    din("fb", (128, 4), F32)
    din("inpwT", (515, 512), BF16)
    din("inpb", (128, 4), F32)
    din("lnw", (NL, 128, 4), F32)
    din("lnb", (NL, 128, 4), F32)
    din("ipwT", (NL, 512, 2048), BF16)
    din("convw", (NL, 128, 32), F32)
    din("convb", (NL, 128, 8), F32)
    din("xpwT", (NL, 128, 512), BF16)
    din("dtwT_b", (NL, 33, DI), F32)
    din("opwT", (NL, 128, 4096), BF16)
    din("Dp", (NL, 128, 8), F32)
    din("triT_f32", (128, 128), F32)
    din("triT_bf", (128, 128), BF16)
    din("ident_f32", (128, 128), F32)
    din("identz_bf", (128, 128), BF16)
    io["h_last"] = nc.dram_tensor("h_last", [B_LOC, EMB], F32, kind="ExternalOutput").ap()
    return io


def build_kernel(nc, io, an, w00, b0v):
    """Emit the full per-core program."""
    ctx = ExitStack()
    tc = ctx.enter_context(tile.TileContext(nc, pool_alloc_mode="queue"))

    consts = ctx.enter_context(tc.tile_pool(name="consts", bufs=1))
    wpool = ctx.enter_context(tc.tile_pool(name="wpool", bufs=1))
    xipool = ctx.enter_context(tc.tile_pool(name="xipool", bufs=1))
    dram = ctx.enter_context(tc.tile_pool(name="dram", bufs=1, space="DRAM"))
    psq = ctx.enter_context(tc.tile_pool(name="psq", bufs=2, space="PSUM"))
    psy = ctx.enter_context(tc.tile_pool(name="psy", bufs=1, space="PSUM"))

    h_dram = [dram.tile([EMB, L], F32, name=f"h_dram{b}", tag=f"h_dram{b}") for b in range(B_LOC)]
    xc_dram = [dram.tile([DI, L + 1], BF16, name=f"xc_dram{b}", tag=f"xc_dram{b}") for b in range(B_LOC)]
    sz_dram = [dram.tile([DI, L], BF16, name=f"sz_dram{b}", tag=f"sz_dram{b}") for b in range(B_LOC)]
    y_dram = [dram.tile([DI, L], BF16, name=f"y_dram{b}", tag=f"y_dram{b}") for b in range(B_LOC)]

    triT_bf = consts.tile([128, 128], BF16, name="triT_bf")
    nc.sync.dma_start(out=triT_bf, in_=io["triT_bf"])
    triT_f32 = consts.tile([128, 128], F32, name="triT_f32")
    nc.sync.dma_start(out=triT_f32, in_=io["triT_f32"])
    ident_f32 = consts.tile([128, 128], F32, name="ident_f32")
    nc.sync.dma_start(out=ident_f32, in_=io["ident_f32"])
    identz_bf = consts.tile([128, 128], BF16, name="identz_bf")
    nc.sync.dma_start(out=identz_bf, in_=io["identz_bf"])
    ones128 = consts.tile([128, 128], F32, name="ones128")
    nc.vector.memset(ones128, 1.0)
    eps_col = consts.tile([128, 1], F32, name="eps_col")
    nc.vector.memset(eps_col, 1e-5)
    b0_col = consts.tile([1, 1], F32, name="b0_col")
    nc.vector.memset(b0_col, float(b0v))
    fwT_sb = consts.tile([1, 511], F32, name="fwT_sb")
    nc.sync.dma_start(out=fwT_sb, in_=io["fwT"])
    fb_sb = consts.tile([128, 4], F32, name="fb_sb")
    nc.sync.dma_start(out=fb_sb, in_=io["fb"])

    # =====================================================================
    # Embedding -> h_dram[b]
    # =====================================================================
    with tc.tile_pool(name="epool", bufs=1) as epool:
        inpwT_sb = []
        for kb in range(5):
            k0, k1 = kb * 128, min(515, (kb + 1) * 128)
            t = epool.tile([k1 - k0, 512], BF16, name=f"inpwT{kb}", tag=f"inpwT{kb}")
            nc.sync.dma_start(out=t, in_=io["inpwT"][k0:k1, :])
            inpwT_sb.append(t)
        inpb_sb = epool.tile([128, 4], F32, name="inpb_sb", tag="inpb_sb")
        nc.sync.dma_start(out=inpb_sb, in_=io["inpb"])
        for b in range(B_LOC):
            trow = epool.tile([1, L + 1], F32, name="trow", tag="trow")
            nc.vector.memset(trow[:, 0:1], 0.0)
            nc.sync.dma_start(out=trow[:, 1:L + 1], in_=io["xT"][b, 2:3, :])
            featk = []
            for kb in range(5):
                kn = min(515, (kb + 1) * 128) - kb * 128
                featk.append(epool.tile([kn, L], BF16, name=f"feat{kb}", tag=f"feat{kb}"))
            # rows 0..510 = v_per ; 511 = v_lin ; 512,513 = s ; 514 = delta
            for kb in range(4):
                fr0, fr1 = kb * 128, min(511, (kb + 1) * 128)
                rn = fr1 - fr0
                for ts4 in range(4):
                    ps = psq.tile([rn, 512], F32, name="emb_ps", tag="q")
                    nc.tensor.matmul(ps, fwT_sb[:, fr0:fr1],
                                     trow[:, 1 + ts4 * 512:1 + (ts4 + 1) * 512],
                                     start=True, stop=True)
                    nc.scalar.activation(featk[kb][0:rn, ts4 * 512:(ts4 + 1) * 512],
                                         ps, AF.Sin, bias=fb_sb[0:rn, kb:kb + 1])
            stage_v = epool.tile([1, L], BF16, name="stage_v", tag="stage_v")
            nc.scalar.activation(stage_v, trow[:, 1:L + 1], AF.Identity,
                                 scale=float(w00), bias=b0_col)   # v_lin
            stage_d = epool.tile([1, L], BF16, name="stage_d", tag="stage_d")
            nc.vector.tensor_tensor(out=stage_d, in0=trow[:, 1:L + 1],
                                    in1=trow[:, 0:L], op=OP.subtract)  # delta
            nc.sync.dma_start(out=featk[3][127:128, :], in_=stage_v)
            nc.sync.dma_start(out=featk[4][2:3, :], in_=stage_d)
            s01 = epool.tile([2, L], F32, name="s01", tag="s01")
            nc.sync.dma_start(out=s01, in_=io["xT"][b, 0:2, :])
            nc.vector.tensor_copy(featk[4][0:2, :], s01)
            for eb in range(EBLK):
                hblk = epool.tile([128, L], F32, name="h0blk", tag="hblk0")
                for ts4 in range(4):
                    sl = slice(ts4 * 512, (ts4 + 1) * 512)
                    ps = psq.tile([128, 512], F32, name="h0ps", tag="q")
                    for kb in range(5):
                        nc.tensor.matmul(ps, inpwT_sb[kb][:, eb * 128:(eb + 1) * 128],
                                         featk[kb][:, sl], start=(kb == 0), stop=(kb == 4))
                    nc.scalar.activation(hblk[:, sl], ps, AF.Identity,
                                         bias=inpb_sb[:, eb:eb + 1])
                nc.sync.dma_start(out=h_dram[b][eb * 128:(eb + 1) * 128, :], in_=hblk)

    # =====================================================================
    # Layers
    # =====================================================================
    for l in range(NL):
        ipwT = []
        for kb in range(EBLK):
            t = wpool.tile([128, 2048], BF16, name=f"ipwT{kb}", tag=f"ipwT{kb}")
            nc.sync.dma_start(out=t, in_=io["ipwT"][l, kb * 128:(kb + 1) * 128, :])
            ipwT.append(t)
        opwT = wpool.tile([128, DBLK * 512], BF16, name="opwT", tag="opwT")
        nc.sync.dma_start(out=opwT, in_=io["opwT"][l])
        xpwT = wpool.tile([128, DBLK * 64], BF16, name="xpwT", tag="xpwT")
        nc.sync.dma_start(out=xpwT, in_=io["xpwT"][l])
        dtwT = wpool.tile([33, DI], F32, name="dtwT", tag="dtwT")
        nc.sync.dma_start(out=dtwT, in_=io["dtwT_b"][l, :, :])
        lnwb = wpool.tile([128, 8], F32, name="lnwb", tag="lnwb")   # cols 0-3 w, 4-7 b
        nc.sync.dma_start(out=lnwb[:, 0:4], in_=io["lnw"][l])
        nc.sync.dma_start(out=lnwb[:, 4:8], in_=io["lnb"][l])
        convw = wpool.tile([128, 32], F32, name="convw", tag="convw")
        nc.sync.dma_start(out=convw, in_=io["convw"][l])
        cbd = wpool.tile([128, 16], F32, name="cbd", tag="cbd")     # cols 0-7 convb, 8-15 D
        nc.sync.dma_start(out=cbd[:, 0:8], in_=io["convb"][l])
        nc.sync.dma_start(out=cbd[:, 8:16], in_=io["Dp"][l])

        with tc.tile_pool(name="midp", bufs=1) as midp:
            dtr_pad = []
            bc_pad = []
            for b in range(B_LOC):
                dp = midp.tile([33, L + 1], F32, name=f"dtr_pad{b}", tag=f"dtr_pad{b}")
                bp = midp.tile([32, L + 1], F32, name=f"bc_pad{b}", tag=f"bc_pad{b}")
                nc.vector.memset(dp[32:33, :], 1.0)
                nc.vector.memset(dp[0:32, 0:1], 0.0)   # col 0 never written below
                nc.vector.memset(bp[:, 0:1], 0.0)      # col 0 garbage -> NaN via 0*inf
                dtr_pad.append(dp)
                bc_pad.append(bp)
            xc_sbuf = {b: [] for b in range(B_LOC)}

            # ============ phase A (per sequence) ============
            for b in range(B_LOC):
                with tc.tile_pool(name="apool", bufs=1) as apool, \
                     tc.tile_pool(name="hstr", bufs=2) as hstr:
                    mu_bc = apool.tile([128, L], F32, name="mu_bc", tag="mu_bc")
                    rstd_bc = apool.tile([128, L], F32, name="rstd_bc", tag="rstd_bc")
                    pm = [psq.tile([128, 1024], F32, name=f"pm{i}", tag="q") for i in range(2)]
                    for eb in range(EBLK):
                        hb = hstr.tile([128, L], F32, name="h_in", tag="hblk")
                        nc.sync.dma_start(out=hb, in_=h_dram[b][eb * 128:(eb + 1) * 128, :])
                        for h2 in range(2):
                            for q2 in range(2):
                                nc.tensor.matmul(pm[h2][:, q2 * 512:(q2 + 1) * 512], ones128,
                                                 hb[:, h2 * 1024 + q2 * 512:h2 * 1024 + (q2 + 1) * 512],
                                                 start=(eb == 0), stop=(eb == EBLK - 1))
                    for h2 in range(2):
                        nc.scalar.activation(mu_bc[:, h2 * 1024:(h2 + 1) * 1024], pm[h2],
                                             AF.Copy, scale=1.0 / EMB)
                    pm2 = [psq.tile([128, 1024], F32, name=f"pm2{i}", tag="q") for i in range(2)]
                    for eb in range(EBLK):
                        hb = hstr.tile([128, L], F32, name="h_in2", tag="hblk")
                        nc.sync.dma_start(out=hb, in_=h_dram[b][eb * 128:(eb + 1) * 128, :])
                        sqs = apool.tile([128, L], F32, name="sqs", tag="scratch8k")
                        nc.vector.tensor_tensor(out=sqs, in0=hb, in1=hb, op=OP.mult)
                        for h2 in range(2):
                            for q2 in range(2):
                                nc.tensor.matmul(pm2[h2][:, q2 * 512:(q2 + 1) * 512], ones128,
                                                 sqs[:, h2 * 1024 + q2 * 512:h2 * 1024 + (q2 + 1) * 512],
                                                 start=(eb == 0), stop=(eb == EBLK - 1))
                    mu2 = apool.tile([128, L], F32, name="mu2", tag="scratch8k")
                    nc.vector.tensor_tensor(out=mu2, in0=mu_bc, in1=mu_bc, op=OP.mult)
                    for h2 in range(2):
                        sl2 = slice(h2 * 1024, (h2 + 1) * 1024)
                        nc.vector.scalar_tensor_tensor(out=rstd_bc[:, sl2], in0=pm2[h2],
                                                       scalar=1.0 / EMB, in1=mu2[:, sl2],
                                                       op0=OP.mult, op1=OP.subtract)
                    nc.scalar.activation(rstd_bc, rstd_bc, AF.Ln, bias=eps_col)
                    nc.scalar.activation(rstd_bc, rstd_bc, AF.Exp, scale=-0.5)
                    hn = []
                    for eb in range(EBLK):
                        hb = hstr.tile([128, L], F32, name="h_in3", tag="hblk")
                        nc.sync.dma_start(out=hb, in_=h_dram[b][eb * 128:(eb + 1) * 128, :])
                        t1 = apool.tile([128, L], F32, name="lnt1", tag="scratch8k")
                        nc.vector.tensor_tensor(out=t1, in0=hb, in1=mu_bc, op=OP.subtract)
                        nc.vector.tensor_tensor(out=t1, in0=t1, in1=rstd_bc, op=OP.mult)
                        hnb = apool.tile([128, L], BF16, name=f"hn{eb}", tag=f"hn{eb}")
                        nc.scalar.activation(hnb, t1, AF.Identity,
                                             scale=lnwb[:, eb:eb + 1], bias=lnwb[:, 4 + eb:5 + eb])
                        hn.append(hnb)
                    # ---- in_proj ----
                    xi_blocks = []
                    for ob in range(16):
                        is_x = ob < 8
                        if is_x:
                            dst = xipool.tile([128, L + DC - 1], BF16, name=f"xi{ob}",
                                              tag=f"xi{ob % 8}")
                            nc.vector.memset(dst[:, 0:DC - 1], 0.0)
                            xi_blocks.append(dst)
                        else:
                            dst = apool.tile([128, L], BF16, name="zblk", tag="zblk")
                        for ts4 in range(4):
                            sl = slice(ts4 * 512, (ts4 + 1) * 512)
                            ps = psq.tile([128, 512], F32, name="ip_ps", tag="q")
                            for kb in range(EBLK):
                                nc.tensor.matmul(ps, ipwT[kb][:, ob * 128:(ob + 1) * 128],
                                                 hn[kb][:, sl], start=(kb == 0), stop=(kb == EBLK - 1))
                            if is_x:
                                nc.scalar.activation(dst[:, DC - 1 + ts4 * 512:DC - 1 + (ts4 + 1) * 512],
                                                     ps, AF.Copy)
                            else:
                                nc.scalar.activation(dst[:, sl], ps, AF.Silu)
                        if not is_x:
                            nc.sync.dma_start(out=sz_dram[b][(ob - 8) * 128:(ob - 7) * 128, :], in_=dst)
                    # ---- conv ----
                    for db in range(DBLK):
                        xi = xi_blocks[db]
                        t_a = apool.tile([128, L], F32, name="t_a", tag="scratch8k")
                        nc.vector.tensor_scalar_mul(t_a, xi[:, 0:L], convw[:, db * 4:db * 4 + 1])
                        for k in range(1, DC):
                            nc.vector.scalar_tensor_tensor(
                                out=t_a, in0=xi[:, k:k + L],
                                scalar=convw[:, db * 4 + k:db * 4 + k + 1],
                                in1=t_a, op0=OP.mult, op1=OP.add)
                        xcb = xipool.tile([128, L], BF16, name=f"xc{db}", tag=f"xi{db}")
                        nc.scalar.activation(xcb, t_a, AF.Silu, bias=cbd[:, db:db + 1])
                        nc.sync.dma_start(out=xc_dram[b][db * 128:(db + 1) * 128, 1:L + 1], in_=xcb)
                        xc_sbuf[b].append(xcb)
                    # ---- x_proj ----
                    for ts4 in range(4):
                        sl = slice(ts4 * 512, (ts4 + 1) * 512)
                        slp = slice(1 + ts4 * 512, 1 + (ts4 + 1) * 512)
                        ps = psq.tile([64, 512], F32, name="xp_ps", tag="q")
                        for kb in range(DBLK):
                            nc.tensor.matmul(ps, xpwT[:, kb * 64:(kb + 1) * 64],
                                             xc_sbuf[b][kb][:, sl],
                                             start=(kb == 0), stop=(kb == DBLK - 1))
                        nc.scalar.activation(dtr_pad[b][0:32, slp], ps[0:32, :], AF.Copy)
                        nc.scalar.activation(bc_pad[b][:, slp], ps[32:64, :], AF.Copy)

            # ============ scan: both sequences interleaved chunk-by-chunk ====
            with tc.tile_pool(name="sp1", bufs=1) as sp1, \
                 tc.tile_pool(name="sp2", bufs=2) as sp2, \
                 tc.tile_pool(name="gpool", bufs=1) as gpool:
                carry_sb = []
                for b in range(B_LOC):
                    cs = sp1.tile([NS, DI], BF16, name=f"carry{b}", tag=f"carry{b}")
                    nc.vector.memset(cs, 0.0)
                    carry_sb.append(cs)
                for c in range(NCHUNK):
                    steps = min(CH, L - c * CH)
                    rows = steps + 1
                    full = (steps == CH)
                    for b in range(B_LOC):
                        ps_dt = psq.tile([rows, DI], F32, name="ps_dt", tag="q")
                        lhs_dtr = dtr_pad[b][:, c * CH:c * CH + rows]
                        for h2 in range(2):
                            nc.tensor.matmul(ps_dt[:, h2 * 512:(h2 + 1) * 512],
                                             lhs_dtr, dtwT[:, h2 * 512:(h2 + 1) * 512],
                                             start=True, stop=True)
                        dt_t = sp1.tile([128, DI], F32, name="dt_t", tag=f"dt_t{b}")
                        if not full:
                            nc.vector.memset(dt_t, 0.0)
                        nc.scalar.activation(dt_t[0:rows, :], ps_dt, AF.Exp)
                        nc.scalar.activation(dt_t[0:rows, :], dt_t[0:rows, :], AF.Ln, bias=1.0)
                        nc.gpsimd.memset(dt_t[0:1, :], 0.0)
                        ps_P = psq.tile([128, DI], F32, name="ps_P", tag="q")
                        for h2 in range(2):
                            nc.tensor.matmul(ps_P[:, h2 * 512:(h2 + 1) * 512],
                                             triT_f32, dt_t[:, h2 * 512:(h2 + 1) * 512],
                                             start=True, stop=True)
                        P_sb = sp1.tile([128, DI], F32, name="P_sb", tag=f"P_sb{b}")
                        nc.scalar.activation(P_sb, ps_P, AF.Copy)
                        u_t = sp1.tile([128, DI], BF16, name="u_t", tag=f"u_t{b}")
                        nc.sync.dma_start_transpose(u_t[0:rows, :],
                                                    xc_dram[b][:, c * CH:c * CH + rows])
                        if c == 0:
                            nc.vector.memset(u_t[0:1, :], 0.0)  # xc col 0 is uninit DRAM
                        nc.vector.tensor_tensor(out=u_t[0:rows, :], in0=dt_t[0:rows, :],
                                                in1=u_t[0:rows, :], op=OP.mult)
                        ps_bc = psq.tile([rows, 32], F32, name="ps_bc", tag="q")
                        nc.tensor.transpose(ps_bc, bc_pad[b][:, c * CH:c * CH + rows],
                                            ident_f32[0:32, 0:32])
                        bc_cols = sp1.tile([128, 32], F32, name="bc_cols", tag=f"bc_cols{b}")
                        nc.scalar.activation(bc_cols[0:rows, :], ps_bc, AF.Copy)
                        ps_y = psy.tile([128, DI], F32, name="ps_y", tag=f"psy{b}")
                        for g in range(4):
                            Gg = gpool.tile([128, 4 * DI], BF16, name="Gg", tag=f"Gg{b}")
                            e1g = gpool.tile([128, 4 * DI], BF16, name="e1g", tag=f"epg{b}")
                            if not full:
                                nc.vector.memset(Gg, 0.0)
                                nc.vector.memset(e1g, 0.0)
                            for j in range(4):
                                n = g * 4 + j
                                a_n = float(an[l, n])
                                dsl = slice(j * DI, (j + 1) * DI)
                                E1p = sp2.tile([128, DI], BF16, name="E1p", tag="E1p")
                                nc.scalar.activation(E1p[0:rows, :], P_sb[0:rows, :],
                                                     AF.Exp, scale=a_n)
                                nc.scalar.activation(e1g[0:rows, dsl], P_sb[0:rows, :],
                                                     AF.Exp, scale=-a_n)
                                bu = sp2.tile([128, DI], BF16, name="bu", tag="bu")
                                nc.vector.tensor_scalar_mul(bu[0:rows, :], u_t[0:rows, :],
                                                            bc_cols[0:rows, n:n + 1])
                                # gpsimd elementwise is ~2.5us/op: give it one n per
                                # group, vector the rest
                                geng = nc.gpsimd if j == 0 else nc.vector
                                geng.tensor_tensor(out=Gg[0:rows, dsl],
                                                   in0=E1p[0:rows, :],
                                                   in1=bu[0:rows, :], op=OP.mult)
                            nc.gpsimd.dma_start(out=Gg[0:1, :],
                                                in_=carry_sb[b][g * 4:(g + 1) * 4, :])
                            for j in range(4):
                                n = g * 4 + j
                                dsl = slice(j * DI, (j + 1) * DI)
                                ps_q = psq.tile([128, DI], F32, name="ps_q", tag="q")
                                for h2 in range(2):
                                    nc.tensor.matmul(ps_q[:, h2 * 512:(h2 + 1) * 512], triT_bf,
                                                     Gg[:, j * DI + h2 * 512:j * DI + (h2 + 1) * 512],
                                                     start=True, stop=True)
                                nc.vector.tensor_tensor(out=Gg[:, dsl], in0=ps_q,
                                                        in1=e1g[:, dsl], op=OP.mult)
                                diag = sp2.tile([128, 128], BF16, name="diag", tag="diag")
                                nc.vector.tensor_scalar_mul(diag, identz_bf,
                                                            bc_cols[:, 16 + n:17 + n])
                                for h2 in range(2):
                                    nc.tensor.matmul(ps_y[:, h2 * 512:(h2 + 1) * 512], diag,
                                                     Gg[:, j * DI + h2 * 512:j * DI + (h2 + 1) * 512],
                                                     start=(n == 0), stop=(n == NS - 1),
                                                     skip_group_check=True)
                            if c < NCHUNK - 1:
                                nc.gpsimd.dma_start(out=carry_sb[b][g * 4:(g + 1) * 4, :],
                                                    in_=Gg[CH:CH + 1, :])
                        y_sb = sp1.tile([128, DI], F32, name="y_sb", tag=f"y_sb{b}")
                        nc.scalar.activation(y_sb, ps_y, AF.Copy)
                        for db in range(DBLK):
                            ps_t = psq.tile([128, 128], F32, name="ps_t", tag="q")
                            nc.tensor.transpose(ps_t, y_sb[:, db * 128:(db + 1) * 128], ident_f32)
                            ytile = sp2.tile([128, CH], BF16, name="ytile", tag="ytile")
                            nc.scalar.activation(ytile[:, 0:steps], ps_t[:, 1:rows], AF.Copy)
                            nc.scalar.dma_start(
                                out=y_dram[b][db * 128:(db + 1) * 128, c * CH:c * CH + steps],
                                in_=ytile[:, 0:steps])

            # ============ epilogue (per sequence, in L-halves) ============
            for b in range(B_LOC):
                with tc.tile_pool(name="epi", bufs=1) as epi:
                    LH = L // 2
                    for lh in range(2):
                        c0 = lh * LH
                        y2b = []
                        for db in range(DBLK):
                            yb = epi.tile([128, LH], BF16, name="yb", tag=f"yb{db % 2}")
                            nc.sync.dma_start(out=yb,
                                              in_=y_dram[b][db * 128:(db + 1) * 128, c0:c0 + LH])
                            szb = epi.tile([128, LH], BF16, name="szb", tag=f"szb{db % 2}")
                            nc.sync.dma_start(out=szb,
                                              in_=sz_dram[b][db * 128:(db + 1) * 128, c0:c0 + LH])
                            xcb_e = epi.tile([128, LH], BF16, name="xcb_e", tag=f"xcb_e{db % 2}")
                            nc.sync.dma_start(out=xcb_e,
                                              in_=xc_dram[b][db * 128:(db + 1) * 128,
                                                             1 + c0:1 + c0 + LH])
                            y2 = epi.tile([128, LH], BF16, name=f"y2_{db}", tag=f"y2_{db}")
                            nc.vector.scalar_tensor_tensor(out=y2, in0=xcb_e,
                                                           scalar=cbd[:, 8 + db:9 + db],
                                                           in1=yb, op0=OP.mult, op1=OP.add)
                            nc.vector.tensor_tensor(out=y2, in0=y2, in1=szb, op=OP.mult)
                            y2b.append(y2)
                        for eb in range(EBLK):
                            hb2 = epi.tile([128, LH], F32, name="h_out", tag=f"h_out{eb % 2}")
                            nc.sync.dma_start(out=hb2,
                                              in_=h_dram[b][eb * 128:(eb + 1) * 128, c0:c0 + LH])
                            for ts2 in range(2):
                                sl = slice(ts2 * 512, (ts2 + 1) * 512)
                                ps = psq.tile([128, 512], F32, name="op_ps", tag="q")
                                for kb in range(DBLK):
                                    nc.tensor.matmul(ps, opwT[:, kb * 512 + eb * 128:kb * 512 + (eb + 1) * 128],
                                                     y2b[kb][:, sl], start=(kb == 0), stop=(kb == DBLK - 1))
                                nc.vector.tensor_tensor(out=hb2[:, sl], in0=hb2[:, sl],
                                                        in1=ps, op=OP.add)
                            nc.sync.dma_start(out=h_dram[b][eb * 128:(eb + 1) * 128, c0:c0 + LH],
                                              in_=hb2)

    for b in range(B_LOC):
        nc.sync.dma_start(out=io["h_last"][b:b + 1, :],
                          in_=h_dram[b][:, L - 1:L].rearrange("e o -> o e"))

    ctx.close()
    return {}


_CACHE = {}


def _install_ntff_shim():
    import sys, types
    if "antenv.axon_hooks" in sys.modules:
        return
    try:
        mod = types.ModuleType("antenv.axon_hooks")
        mod._hook = None
        def set_axon_ntff_profile_hook(h): mod._hook = h
        def get_axon_ntff_profile_hook(): return mod._hook
        mod.set_axon_ntff_profile_hook = set_axon_ntff_profile_hook
        mod.get_axon_ntff_profile_hook = get_axon_ntff_profile_hook
        import antenv
        antenv.axon_hooks = mod
        sys.modules["antenv.axon_hooks"] = mod
        from trn_agent_boot.trn_boot import _ntff_profile_via_ctypes
        hook = _ntff_profile_via_ctypes("/opt/axon/libaxon_pjrt.so")
        set_axon_ntff_profile_hook(hook)
    except Exception:
        pass


def _get_program(an, w00, b0v):
    key = (tuple(np.asarray(an).ravel().tolist()), float(w00), float(b0v))
    if key in _CACHE:
        return _CACHE[key]
    nc = bacc.Bacc("TRN2", target_bir_lowering=False, debug=False, num_devices=8)
    io = declare_io(nc)
    build_kernel(nc, io, an, w00, b0v)
    nc.compile()
    _CACHE[key] = nc
    return nc


def _softplus(x):
    return np.log1p(np.exp(-np.abs(x))) + np.maximum(x, 0)


def kernel(**inputs):
    trace = bool(int(os.environ.get("BASS_KERNEL_TRACE", "0")))
    if trace:
        _install_ntff_shim()
    an = an_scales(inputs)
    w00 = float(np.asarray(inputs["t2v_lin_w"])[0, 0])
    b0v = float(np.asarray(inputs["t2v_lin_b"])[0])
    nc = _get_program(an, w00, b0v)
    in_maps = [host_prep(inputs, c) for c in range(8)]
    res = run_bass_kernel_spmd(nc, in_maps, core_ids=list(range(8)), trace=trace)
    if trace and res.exec_time_ns is not None:
        print(f"HW exec time: {res.exec_time_ns} ns")
        kernel.last_exec_time_ns = res.exec_time_ns
    h_last = np.concatenate([r["h_last"] for r in res.results], axis=0)  # [16, 512]
    dec_w = np.asarray(inputs["dec_w"], np.float32)
    dec_b = np.asarray(inputs["dec_b"], np.float32)
    stats = h_last.astype(np.float32) @ dec_w.T + dec_b
    m, v_ = np.split(stats, 2, axis=-1)
    v = _softplus(v_) + 1e-5
    return (m.astype(np.float32), v.astype(np.float32))


# revision 16
# speedup vs baseline: 1.6851x; 1.2139x over previous
"""Trainium2 Bass kernel for the Mamba-style encoder problem.

Self-contained: builds and runs an 8-core SPMD Bass program, data-parallel
over the batch (2 sequences per core). Returns (m, v) like the reference.
"""
import os
import numpy as np
import ml_dtypes
from contextlib import ExitStack

import concourse.bass as bass
import concourse.bacc as bacc
import concourse.tile as tile
from concourse import mybir
from concourse.bass_utils import run_bass_kernel_spmd

F32 = mybir.dt.float32
BF16 = mybir.dt.bfloat16
AF = mybir.ActivationFunctionType
OP = mybir.AluOpType

B_LOC, L, EMB = 2, 2048, 512
NL, NS, DI, DC, DR = 4, 16, 1024, 4, 32
CH = 127                 # time steps per chunk
NCHUNK = (L + CH - 1) // CH   # 17 (last chunk 2048-16*127=16 steps)
DBLK = DI // 128         # 8
EBLK = EMB // 128        # 4
NGRP = 4                 # n's per PSUM group (16 states / 4 groups)


def bf(x):
    return np.ascontiguousarray(x).astype(ml_dtypes.bfloat16)


def host_prep(inputs, core_id):
    """Build the per-core in_map (numpy) from full inputs."""
    d = inputs
    b0 = core_id * B_LOC
    x = np.asarray(d["x"][b0:b0 + B_LOC])          # [2, 2048, 3]
    m = {}
    m["xT"] = np.ascontiguousarray(x.transpose(0, 2, 1)).astype(np.float32)  # [2,3,2048]
    m["fwT"] = np.asarray(d["t2v_freq_w"]).reshape(1, 511).astype(np.float32)
    fb = np.asarray(d["t2v_freq_b"]).astype(np.float32)
    fbb = np.zeros((128, 4), np.float32)
    for kb in range(4):
        fr0 = kb * 128; fr1 = min(511, (kb + 1) * 128)
        fbb[: fr1 - fr0, kb] = fb[fr0:fr1]
    m["fb"] = fbb
    iw = np.asarray(d["inp_w"])                      # [512, 515] cols: [s0,s1,delta,vlin,vper...]
    perm = list(range(4, 515)) + [3, 0, 1, 2]        # -> [vper(511), vlin, s0, s1, delta]
    m["inpwT"] = bf(iw[:, perm].T)                   # [515, 512]
    m["inpb"] = np.asarray(d["inp_b"]).reshape(4, 128).T.astype(np.float32).copy()
    m["lnw"] = np.ascontiguousarray(np.asarray(d["ln_w"]).reshape(NL, 4, 128).transpose(0, 2, 1)).astype(np.float32)
    m["lnb"] = np.ascontiguousarray(np.asarray(d["ln_b"]).reshape(NL, 4, 128).transpose(0, 2, 1)).astype(np.float32)
    m["ipwT"] = bf(np.asarray(d["in_proj_w"]).transpose(0, 2, 1))   # [4, 512, 2048]
    cw = np.asarray(d["conv_w"]).astype(np.float32)                  # [4, 1024, 4]
    m["convw"] = np.ascontiguousarray(cw.reshape(NL, 8, 128, 4).transpose(0, 2, 1, 3).reshape(NL, 128, 32))
    m["convb"] = np.ascontiguousarray(np.asarray(d["conv_b"]).reshape(NL, 8, 128).transpose(0, 2, 1)).astype(np.float32)
    xpt = np.asarray(d["x_proj_w"]).transpose(0, 2, 1)               # [4, 1024, 64]
    m["xpwT"] = bf(xpt.reshape(NL, 8, 128, 64).transpose(0, 2, 1, 3).reshape(NL, 128, 512))
    dtw = np.asarray(d["dt_w"])                                      # [4, 1024, 32]
    dtb = np.asarray(d["dt_b"])                                      # [4, 1024]
    dtwT_b = np.concatenate([dtw.transpose(0, 2, 1),
                             dtb[:, None, :]], axis=1)               # [4, 33, 1024]
    m["dtwT_b"] = dtwT_b.astype(np.float32)
    opt = np.asarray(d["out_proj_w"]).transpose(0, 2, 1)             # [4, 1024, 512]
    m["opwT"] = bf(opt.reshape(NL, 8, 128, 512).transpose(0, 2, 1, 3).reshape(NL, 128, 4096))
    m["Dp"] = np.ascontiguousarray(np.asarray(d["D"]).reshape(NL, 8, 128).transpose(0, 2, 1)).astype(np.float32)
    tri = np.tril(np.ones((128, 128), np.float32))
    m["triT_f32"] = np.ascontiguousarray(tri.T)      # lhsT for cumsum [s,t] upper-tri
    m["triT_bf"] = bf(tri.T)
    ident = np.eye(128, dtype=np.float32)
    m["ident_f32"] = ident
    identz = ident.copy(); identz[0, 0] = 0.0        # row-0-zeroed identity
    m["identz_bf"] = bf(identz)
    return m


def an_scales(inputs):
    """Per (layer, n) decay magnitudes a_n = -A[l,0,n]; assert d-uniform."""
    A_log = np.asarray(inputs["A_log"])  # [NL, DI, NS]
    A = -np.exp(A_log.astype(np.float64))
    spread = np.abs(A - A[:, :1, :]).max()
    assert spread < 1e-5 * max(1.0, np.abs(A).max()), \
        f"A_log not d-uniform (spread {spread}); kernel assumes per-n scalar decay"
    return (-A[:, 0, :]).astype(np.float64)   # [NL, NS] positive magnitudes


def declare_io(nc):
    io = {}
    def din(name, shape, dt):
        io[name] = nc.dram_tensor(name, list(shape), dt, kind="ExternalInput").ap()
    din("xT", (B_LOC, 3, L), F32)
    din("fwT", (1, 511), F32)
    din("fb", (128, 4), F32)
    din("inpwT", (515, 512), BF16)
    din("inpb", (128, 4), F32)
    din("lnw", (NL, 128, 4), F32)
    din("lnb", (NL, 128, 4), F32)
    din("ipwT", (NL, 512, 2048), BF16)
    din("convw", (NL, 128, 32), F32)
    din("convb", (NL, 128, 8), F32)
    din("xpwT", (NL, 128, 512), BF16)
    din("dtwT_b", (NL, 33, DI), F32)
    din("opwT", (NL, 128, 4096), BF16)
    din("Dp", (NL, 128, 8), F32)
    din("triT_f32", (128, 128), F32)
    din("triT_bf", (128, 128), BF16)
    din("ident_f32", (128, 128), F32)
    din("identz_bf", (128, 128), BF16)
    io["h_last"] = nc.dram_tensor("h_last", [B_LOC, EMB], F32, kind="ExternalOutput").ap()
    return io


def build_kernel(nc, io, an, w00, b0v):
    """Emit the full per-core program."""
    ctx = ExitStack()
    tc = ctx.enter_context(tile.TileContext(nc, pool_alloc_mode="queue"))

    consts = ctx.enter_context(tc.tile_pool(name="consts", bufs=1))
    wpool = ctx.enter_context(tc.tile_pool(name="wpool", bufs=1))
    xipool = ctx.enter_context(tc.tile_pool(name="xipool", bufs=1))
    dram = ctx.enter_context(tc.tile_pool(name="dram", bufs=1, space="DRAM"))
    psq = ctx.enter_context(tc.tile_pool(name="psq", bufs=3, space="PSUM"))
    psy = ctx.enter_context(tc.tile_pool(name="psy", bufs=1, space="PSUM"))

    h_dram = [dram.tile([EMB, L], F32, name=f"h_dram{b}", tag=f"h_dram{b}") for b in range(B_LOC)]
    xc_dram = [dram.tile([DI, L + 1], BF16, name=f"xc_dram{b}", tag=f"xc_dram{b}") for b in range(B_LOC)]
    sz_dram = [dram.tile([DI, L], BF16, name=f"sz_dram{b}", tag=f"sz_dram{b}") for b in range(B_LOC)]
    y_dram = [dram.tile([DI, L], BF16, name=f"y_dram{b}", tag=f"y_dram{b}") for b in range(B_LOC)]

    triT_bf = consts.tile([128, 128], BF16, name="triT_bf")
    nc.sync.dma_start(out=triT_bf, in_=io["triT_bf"])
    triT_f32 = consts.tile([128, 128], F32, name="triT_f32")
    nc.sync.dma_start(out=triT_f32, in_=io["triT_f32"])
    ident_f32 = consts.tile([128, 128], F32, name="ident_f32")
    nc.sync.dma_start(out=ident_f32, in_=io["ident_f32"])
    identz_bf = consts.tile([128, 128], BF16, name="identz_bf")
    nc.sync.dma_start(out=identz_bf, in_=io["identz_bf"])
    ones128 = consts.tile([128, 128], F32, name="ones128")
    nc.vector.memset(ones128, 1.0)
    eps_col = consts.tile([128, 1], F32, name="eps_col")
    nc.vector.memset(eps_col, 1e-5)
    b0_col = consts.tile([1, 1], F32, name="b0_col")
    nc.vector.memset(b0_col, float(b0v))
    fwT_sb = consts.tile([1, 511], F32, name="fwT_sb")
    nc.sync.dma_start(out=fwT_sb, in_=io["fwT"])
    fb_sb = consts.tile([128, 4], F32, name="fb_sb")
    nc.sync.dma_start(out=fb_sb, in_=io["fb"])

    # =====================================================================
    # Embedding -> h_dram[b]
    # =====================================================================
    with tc.tile_pool(name="epool", bufs=1) as epool:
        inpwT_sb = []
        for kb in range(5):
            k0, k1 = kb * 128, min(515, (kb + 1) * 128)
            t = epool.tile([k1 - k0, 512], BF16, name=f"inpwT{kb}", tag=f"inpwT{kb}")
            nc.sync.dma_start(out=t, in_=io["inpwT"][k0:k1, :])
            inpwT_sb.append(t)
        inpb_sb = epool.tile([128, 4], F32, name="inpb_sb", tag="inpb_sb")
        nc.sync.dma_start(out=inpb_sb, in_=io["inpb"])
        for b in range(B_LOC):
            trow = epool.tile([1, L + 1], F32, name="trow", tag="trow")
            nc.vector.memset(trow[:, 0:1], 0.0)
            nc.sync.dma_start(out=trow[:, 1:L + 1], in_=io["xT"][b, 2:3, :])
            featk = []
            for kb in range(5):
                kn = min(515, (kb + 1) * 128) - kb * 128
                featk.append(epool.tile([kn, L], BF16, name=f"feat{kb}", tag=f"feat{kb}"))
            # rows 0..510 = v_per ; 511 = v_lin ; 512,513 = s ; 514 = delta
            for kb in range(4):
                fr0, fr1 = kb * 128, min(511, (kb + 1) * 128)
                rn = fr1 - fr0
                for ts4 in range(4):
                    ps = psq.tile([rn, 512], F32, name="emb_ps", tag="q")
                    nc.tensor.matmul(ps, fwT_sb[:, fr0:fr1],
                                     trow[:, 1 + ts4 * 512:1 + (ts4 + 1) * 512],
                                     start=True, stop=True)
                    nc.scalar.activation(featk[kb][0:rn, ts4 * 512:(ts4 + 1) * 512],
                                         ps, AF.Sin, bias=fb_sb[0:rn, kb:kb + 1])
            stage_v = epool.tile([1, L], BF16, name="stage_v", tag="stage_v")
            nc.scalar.activation(stage_v, trow[:, 1:L + 1], AF.Identity,
                                 scale=float(w00), bias=b0_col)   # v_lin
            stage_d = epool.tile([1, L], BF16, name="stage_d", tag="stage_d")
            nc.vector.tensor_tensor(out=stage_d, in0=trow[:, 1:L + 1],
                                    in1=trow[:, 0:L], op=OP.subtract)  # delta
            nc.sync.dma_start(out=featk[3][127:128, :], in_=stage_v)
            nc.sync.dma_start(out=featk[4][2:3, :], in_=stage_d)
            s01 = epool.tile([2, L], F32, name="s01", tag="s01")
            nc.sync.dma_start(out=s01, in_=io["xT"][b, 0:2, :])
            nc.vector.tensor_copy(featk[4][0:2, :], s01)
            for eb in range(EBLK):
                hblk = epool.tile([128, L], F32, name="h0blk", tag="hblk0")
                for ts4 in range(4):
                    sl = slice(ts4 * 512, (ts4 + 1) * 512)
                    ps = psq.tile([128, 512], F32, name="h0ps", tag="q")
                    for kb in range(5):
                        nc.tensor.matmul(ps, inpwT_sb[kb][:, eb * 128:(eb + 1) * 128],
                                         featk[kb][:, sl], start=(kb == 0), stop=(kb == 4))
                    nc.scalar.activation(hblk[:, sl], ps, AF.Identity,
                                         bias=inpb_sb[:, eb:eb + 1])
                nc.sync.dma_start(out=h_dram[b][eb * 128:(eb + 1) * 128, :], in_=hblk)

    # =====================================================================
    # Layers
    # =====================================================================
    for l in range(NL):
        ipwT = []
        for kb in range(EBLK):
            t = wpool.tile([128, 2048], BF16, name=f"ipwT{kb}", tag=f"ipwT{kb}")
            nc.sync.dma_start(out=t, in_=io["ipwT"][l, kb * 128:(kb + 1) * 128, :])
            ipwT.append(t)
        opwT = wpool.tile([128, DBLK * 512], BF16, name="opwT", tag="opwT")
        nc.sync.dma_start(out=opwT, in_=io["opwT"][l])
        xpwT = wpool.tile([128, DBLK * 64], BF16, name="xpwT", tag="xpwT")
        nc.sync.dma_start(out=xpwT, in_=io["xpwT"][l])
        dtwT = wpool.tile([33, DI], F32, name="dtwT", tag="dtwT")
        nc.sync.dma_start(out=dtwT, in_=io["dtwT_b"][l, :, :])
        lnwb = wpool.tile([128, 8], F32, name="lnwb", tag="lnwb")   # cols 0-3 w, 4-7 b
        nc.sync.dma_start(out=lnwb[:, 0:4], in_=io["lnw"][l])
        nc.sync.dma_start(out=lnwb[:, 4:8], in_=io["lnb"][l])
        convw = wpool.tile([128, 32], F32, name="convw", tag="convw")
        nc.sync.dma_start(out=convw, in_=io["convw"][l])
        cbd = wpool.tile([128, 16], F32, name="cbd", tag="cbd")     # cols 0-7 convb, 8-15 D
        nc.sync.dma_start(out=cbd[:, 0:8], in_=io["convb"][l])
        nc.sync.dma_start(out=cbd[:, 8:16], in_=io["Dp"][l])

        for b in range(B_LOC):
            with tc.tile_pool(name="midp", bufs=1) as midp:
                dtr_pad = midp.tile([33, L + 1], F32, name="dtr_pad", tag="dtr_pad")
                bc_pad = midp.tile([32, L + 1], F32, name="bc_pad", tag="bc_pad")
                nc.vector.memset(dtr_pad[32:33, :], 1.0)
                nc.vector.memset(dtr_pad[0:32, 0:1], 0.0)  # col 0 never written below
                nc.vector.memset(bc_pad[:, 0:1], 0.0)      # col 0 garbage -> 0*inf NaN
                xc_sbuf = []   # filled in phase-A, consumed by x_proj (within phase-A)

                # ============ phase A ============
                with tc.tile_pool(name="apool", bufs=1) as apool, \
                     tc.tile_pool(name="hstr", bufs=2) as hstr:
                    mu_bc = apool.tile([128, L], F32, name="mu_bc", tag="mu_bc")
                    rstd_bc = apool.tile([128, L], F32, name="rstd_bc", tag="rstd_bc")
                    pm = [psq.tile([128, 1024], F32, name=f"pm{i}", tag="q") for i in range(2)]
                    for eb in range(EBLK):
                        hb = hstr.tile([128, L], F32, name="h_in", tag="hblk")
                        nc.sync.dma_start(out=hb, in_=h_dram[b][eb * 128:(eb + 1) * 128, :])
                        for h2 in range(2):
                            for q2 in range(2):
                                nc.tensor.matmul(pm[h2][:, q2 * 512:(q2 + 1) * 512], ones128,
                                                 hb[:, h2 * 1024 + q2 * 512:h2 * 1024 + (q2 + 1) * 512],
                                                 start=(eb == 0), stop=(eb == EBLK - 1))
                    for h2 in range(2):
                        nc.scalar.activation(mu_bc[:, h2 * 1024:(h2 + 1) * 1024], pm[h2],
                                             AF.Copy, scale=1.0 / EMB)
                    pm2 = [psq.tile([128, 1024], F32, name=f"pm2{i}", tag="q") for i in range(2)]
                    for eb in range(EBLK):
                        hb = hstr.tile([128, L], F32, name="h_in2", tag="hblk")
                        nc.sync.dma_start(out=hb, in_=h_dram[b][eb * 128:(eb + 1) * 128, :])
                        sqs = apool.tile([128, L], F32, name="sqs", tag="scratch8k")
                        nc.vector.tensor_tensor(out=sqs, in0=hb, in1=hb, op=OP.mult)
                        for h2 in range(2):
                            for q2 in range(2):
                                nc.tensor.matmul(pm2[h2][:, q2 * 512:(q2 + 1) * 512], ones128,
                                                 sqs[:, h2 * 1024 + q2 * 512:h2 * 1024 + (q2 + 1) * 512],
                                                 start=(eb == 0), stop=(eb == EBLK - 1))
                    mu2 = apool.tile([128, L], F32, name="mu2", tag="scratch8k")
                    nc.vector.tensor_tensor(out=mu2, in0=mu_bc, in1=mu_bc, op=OP.mult)
                    for h2 in range(2):
                        sl2 = slice(h2 * 1024, (h2 + 1) * 1024)
                        nc.vector.scalar_tensor_tensor(out=rstd_bc[:, sl2], in0=pm2[h2],
                                                       scalar=1.0 / EMB, in1=mu2[:, sl2],
                                                       op0=OP.mult, op1=OP.subtract)
                    nc.scalar.activation(rstd_bc, rstd_bc, AF.Ln, bias=eps_col)
                    nc.scalar.activation(rstd_bc, rstd_bc, AF.Exp, scale=-0.5)
                    hn = []
                    for eb in range(EBLK):
                        hb = hstr.tile([128, L], F32, name="h_in3", tag="hblk")
                        nc.sync.dma_start(out=hb, in_=h_dram[b][eb * 128:(eb + 1) * 128, :])
                        t1 = apool.tile([128, L], F32, name="lnt1", tag="scratch8k")
                        nc.vector.tensor_tensor(out=t1, in0=hb, in1=mu_bc, op=OP.subtract)
                        nc.vector.tensor_tensor(out=t1, in0=t1, in1=rstd_bc, op=OP.mult)
                        hnb = apool.tile([128, L], BF16, name=f"hn{eb}", tag=f"hn{eb}")
                        nc.scalar.activation(hnb, t1, AF.Identity,
                                             scale=lnwb[:, eb:eb + 1], bias=lnwb[:, 4 + eb:5 + eb])
                        hn.append(hnb)
                    # ---- in_proj ----
                    xi_blocks = []
                    for ob in range(16):
                        is_x = ob < 8
                        if is_x:
                            dst = xipool.tile([128, L + DC - 1], BF16, name=f"xi{ob}", tag=f"xi{ob % 8}")
                            nc.vector.memset(dst[:, 0:DC - 1], 0.0)
                            xi_blocks.append(dst)
                        else:
                            dst = apool.tile([128, L], BF16, name="zblk", tag="zblk")
                        for ts4 in range(4):
                            sl = slice(ts4 * 512, (ts4 + 1) * 512)
                            ps = psq.tile([128, 512], F32, name="ip_ps", tag="q")
                            for kb in range(EBLK):
                                nc.tensor.matmul(ps, ipwT[kb][:, ob * 128:(ob + 1) * 128],
                                                 hn[kb][:, sl], start=(kb == 0), stop=(kb == EBLK - 1))
                            if is_x:
                                nc.scalar.activation(dst[:, DC - 1 + ts4 * 512:DC - 1 + (ts4 + 1) * 512],
                                                     ps, AF.Copy)
                            else:
                                nc.scalar.activation(dst[:, sl], ps, AF.Silu)
                        if not is_x:
                            nc.sync.dma_start(out=sz_dram[b][(ob - 8) * 128:(ob - 7) * 128, :], in_=dst)
                    # ---- conv ----
                    for db in range(DBLK):
                        xi = xi_blocks[db]
                        t_a = apool.tile([128, L], F32, name="t_a", tag="scratch8k")
                        nc.vector.tensor_scalar_mul(t_a, xi[:, 0:L], convw[:, db * 4:db * 4 + 1])
                        for k in range(1, DC):
                            nc.vector.scalar_tensor_tensor(
                                out=t_a, in0=xi[:, k:k + L],
                                scalar=convw[:, db * 4 + k:db * 4 + k + 1],
                                in1=t_a, op0=OP.mult, op1=OP.add)
                        xcb = xipool.tile([128, L], BF16, name=f"xc{db}", tag=f"xi{db}")
                        nc.scalar.activation(xcb, t_a, AF.Silu, bias=cbd[:, db:db + 1])
                        nc.sync.dma_start(out=xc_dram[b][db * 128:(db + 1) * 128, 1:L + 1], in_=xcb)
                        xc_sbuf.append(xcb)
                    # ---- x_proj ----
                    for ts4 in range(4):
                        sl = slice(ts4 * 512, (ts4 + 1) * 512)
                        slp = slice(1 + ts4 * 512, 1 + (ts4 + 1) * 512)
                        ps = psq.tile([64, 512], F32, name="xp_ps", tag="q")
                        for kb in range(DBLK):
                            nc.tensor.matmul(ps, xpwT[:, kb * 64:(kb + 1) * 64], xc_sbuf[kb][:, sl],
                                             start=(kb == 0), stop=(kb == DBLK - 1))
                        nc.scalar.activation(dtr_pad[0:32, slp], ps[0:32, :], AF.Copy)
                        nc.scalar.activation(bc_pad[:, slp], ps[32:64, :], AF.Copy)

                # ============ scan ============
                with tc.tile_pool(name="sp1", bufs=1) as sp1, \
                     tc.tile_pool(name="sp2", bufs=2) as sp2, \
                     tc.tile_pool(name="gpool", bufs=2) as gpool:
                    carry_sb = sp1.tile([NS, DI], BF16, name="carry_sb", tag="carry")
                    nc.vector.memset(carry_sb, 0.0)
                    for c in range(NCHUNK):
                        steps = min(CH, L - c * CH)
                        rows = steps + 1
                        full = (steps == CH)
                        ps_dt = psq.tile([rows, DI], F32, name="ps_dt", tag="q")
                        lhs_dtr = dtr_pad[:, c * CH:c * CH + rows]
                        for h2 in range(2):
                            nc.tensor.matmul(ps_dt[:, h2 * 512:(h2 + 1) * 512],
                                             lhs_dtr, dtwT[:, h2 * 512:(h2 + 1) * 512],
                                             start=True, stop=True)
                        dt_t = sp1.tile([128, DI], F32, name="dt_t", tag="dt_t")
                        if not full:
                            nc.vector.memset(dt_t, 0.0)
                        nc.scalar.activation(dt_t[0:rows, :], ps_dt, AF.Exp)
                        nc.scalar.activation(dt_t[0:rows, :], dt_t[0:rows, :], AF.Ln, bias=1.0)
                        nc.gpsimd.memset(dt_t[0:1, :], 0.0)
                        ps_P = psq.tile([128, DI], F32, name="ps_P", tag="q")
                        for h2 in range(2):
                            nc.tensor.matmul(ps_P[:, h2 * 512:(h2 + 1) * 512],
                                             triT_f32, dt_t[:, h2 * 512:(h2 + 1) * 512],
                                             start=True, stop=True)
                        P_sb = sp1.tile([128, DI], F32, name="P_sb", tag="P_sb")
                        nc.scalar.activation(P_sb, ps_P, AF.Copy)
                        u_t = sp1.tile([128, DI], BF16, name="u_t", tag="u_t")
                        nc.sync.dma_start_transpose(u_t[0:rows, :], xc_dram[b][:, c * CH:c * CH + rows])
                        if c == 0:
                            nc.vector.memset(u_t[0:1, :], 0.0)  # xc col 0 is uninit DRAM
                        nc.vector.tensor_tensor(out=u_t[0:rows, :], in0=dt_t[0:rows, :],
                                                in1=u_t[0:rows, :], op=OP.mult)
                        ps_bc = psy.tile([rows, 32], F32, name="ps_bc", tag="y")
                        nc.tensor.transpose(ps_bc, bc_pad[:, c * CH:c * CH + rows], ident_f32[0:32, 0:32])
                        bc_cols = sp1.tile([128, 32], F32, name="bc_cols", tag="bc_cols")
                        nc.scalar.activation(bc_cols[0:rows, :], ps_bc, AF.Copy)
                        ps_y = psy.tile([128, DI], F32, name="ps_y", tag="y")
                        for g in range(4):
                            Gg = gpool.tile([128, 4 * DI], BF16, name="Gg", tag="Gg")
                            e1g = gpool.tile([128, 4 * DI], BF16, name="e1g", tag="e1g")
                            if not full:
                                nc.vector.memset(Gg, 0.0)
                                nc.vector.memset(e1g, 0.0)
                            for j in range(4):
                                n = g * 4 + j
                                a_n = float(an[l, n])
                                dsl = slice(j * DI, (j + 1) * DI)
                                E1p = sp2.tile([128, DI], BF16, name="E1p", tag="E1p")
                                nc.scalar.activation(E1p[0:rows, :], P_sb[0:rows, :], AF.Exp, scale=a_n)
                                nc.scalar.activation(e1g[0:rows, dsl], P_sb[0:rows, :], AF.Exp, scale=-a_n)
                                bu = sp2.tile([128, DI], BF16, name="bu", tag="bu")
                                nc.vector.tensor_scalar_mul(bu[0:rows, :], u_t[0:rows, :],
                                                            bc_cols[0:rows, n:n + 1])
                                nc.vector.tensor_tensor(out=Gg[0:rows, dsl], in0=E1p[0:rows, :],
                                                        in1=bu[0:rows, :], op=OP.mult)
                            nc.gpsimd.dma_start(out=Gg[0:1, :], in_=carry_sb[g * 4:(g + 1) * 4, :])
                            for j in range(4):
                                n = g * 4 + j
                                dsl = slice(j * DI, (j + 1) * DI)
                                ps_q = psq.tile([128, DI], F32, name="ps_q", tag="q")
                                for h2 in range(2):
                                    nc.tensor.matmul(ps_q[:, h2 * 512:(h2 + 1) * 512], triT_bf,
                                                     Gg[:, j * DI + h2 * 512:j * DI + (h2 + 1) * 512],
                                                     start=True, stop=True)
                                # T overwrites the G slice (no longer needed)
                                nc.vector.tensor_tensor(out=Gg[:, dsl], in0=ps_q, in1=e1g[:, dsl],
                                                        op=OP.mult)
                                diag = sp2.tile([128, 128], BF16, name="diag", tag="diag")
                                nc.vector.tensor_scalar_mul(diag, identz_bf, bc_cols[:, 16 + n:17 + n])
                                for h2 in range(2):
                                    nc.tensor.matmul(ps_y[:, h2 * 512:(h2 + 1) * 512], diag,
                                                     Gg[:, j * DI + h2 * 512:j * DI + (h2 + 1) * 512],
                                                     start=(n == 0), stop=(n == NS - 1),
                                                     skip_group_check=True)
                            if c < NCHUNK - 1:
                                nc.gpsimd.dma_start(out=carry_sb[g * 4:(g + 1) * 4, :],
                                                    in_=Gg[CH:CH + 1, :])
                        y_sb = sp1.tile([128, DI], F32, name="y_sb", tag="dt_t")
                        nc.scalar.activation(y_sb, ps_y, AF.Copy)
                        for db in range(DBLK):
                            ps_t = psy.tile([128, 128], F32, name="ps_t", tag="y")
                            nc.tensor.transpose(ps_t, y_sb[:, db * 128:(db + 1) * 128], ident_f32)
                            ytile = sp2.tile([128, CH], BF16, name="ytile", tag="diag")
                            nc.scalar.activation(ytile[:, 0:steps], ps_t[:, 1:rows], AF.Copy)
                            nc.scalar.dma_start(
                                out=y_dram[b][db * 128:(db + 1) * 128, c * CH:c * CH + steps],
                                in_=ytile[:, 0:steps])

                # ============ epilogue ============
                with tc.tile_pool(name="epi", bufs=2) as epi:
                    y2b = []
                    for db in range(DBLK):
                        yb = epi.tile([128, L], BF16, name="yb", tag="yb")
                        nc.sync.dma_start(out=yb, in_=y_dram[b][db * 128:(db + 1) * 128, :])
                        szb = epi.tile([128, L], BF16, name="szb", tag="szb")
                        nc.sync.dma_start(out=szb, in_=sz_dram[b][db * 128:(db + 1) * 128, :])
                        y2 = epi.tile([128, L], BF16, name=f"y2_{db}", tag=f"y2_{db % 4}")
                        nc.vector.scalar_tensor_tensor(out=y2, in0=xc_sbuf[db],
                                                       scalar=cbd[:, 8 + db:9 + db],
                                                       in1=yb, op0=OP.mult, op1=OP.add)
                        nc.vector.tensor_tensor(out=y2, in0=y2, in1=szb, op=OP.mult)
                        y2b.append(y2)
                    for eb in range(EBLK):
                        hb2 = epi.tile([128, L], F32, name="h_out", tag="h_out")
                        nc.sync.dma_start(out=hb2, in_=h_dram[b][eb * 128:(eb + 1) * 128, :])
                        for ts4 in range(4):
                            sl = slice(ts4 * 512, (ts4 + 1) * 512)
                            ps = psq.tile([128, 512], F32, name="op_ps", tag="q")
                            for kb in range(DBLK):
                                nc.tensor.matmul(ps, opwT[:, kb * 512 + eb * 128:kb * 512 + (eb + 1) * 128],
                                                 y2b[kb][:, sl], start=(kb == 0), stop=(kb == DBLK - 1))
                            nc.vector.tensor_tensor(out=hb2[:, sl], in0=hb2[:, sl],
                                                    in1=ps, op=OP.add)
                        nc.sync.dma_start(out=h_dram[b][eb * 128:(eb + 1) * 128, :], in_=hb2)

    for b in range(B_LOC):
        nc.sync.dma_start(out=io["h_last"][b:b + 1, :],
                          in_=h_dram[b][:, L - 1:L].rearrange("e o -> o e"))

    ctx.close()
    return {}


_CACHE = {}


def _install_ntff_shim():
    import sys, types
    if "antenv.axon_hooks" in sys.modules:
        return
    try:
        mod = types.ModuleType("antenv.axon_hooks")
        mod._hook = None
        def set_axon_ntff_profile_hook(h): mod._hook = h
        def get_axon_ntff_profile_hook(): return mod._hook
        mod.set_axon_ntff_profile_hook = set_axon_ntff_profile_hook
        mod.get_axon_ntff_profile_hook = get_axon_ntff_profile_hook
        import antenv
        antenv.axon_hooks = mod
        sys.modules["antenv.axon_hooks"] = mod
        from trn_agent_boot.trn_boot import _ntff_profile_via_ctypes
        hook = _ntff_profile_via_ctypes("/opt/axon/libaxon_pjrt.so")
        set_axon_ntff_profile_hook(hook)
    except Exception:
        pass


def _get_program(an, w00, b0v):
    key = (tuple(np.asarray(an).ravel().tolist()), float(w00), float(b0v))
    if key in _CACHE:
        return _CACHE[key]
    nc = bacc.Bacc("TRN2", target_bir_lowering=False, debug=False, num_devices=8)
    io = declare_io(nc)
    build_kernel(nc, io, an, w00, b0v)
    nc.compile()
    _CACHE[key] = nc
    return nc


def _softplus(x):
    return np.log1p(np.exp(-np.abs(x))) + np.maximum(x, 0)


def kernel(**inputs):
    trace = bool(int(os.environ.get("BASS_KERNEL_TRACE", "0")))
    if trace:
        _install_ntff_shim()
    an = an_scales(inputs)
    w00 = float(np.asarray(inputs["t2v_lin_w"])[0, 0])
    b0v = float(np.asarray(inputs["t2v_lin_b"])[0])
    nc = _get_program(an, w00, b0v)
    in_maps = [host_prep(inputs, c) for c in range(8)]
    res = run_bass_kernel_spmd(nc, in_maps, core_ids=list(range(8)), trace=trace)
    if trace and res.exec_time_ns is not None:
        print(f"HW exec time: {res.exec_time_ns} ns")
        kernel.last_exec_time_ns = res.exec_time_ns
    h_last = np.concatenate([r["h_last"] for r in res.results], axis=0)  # [16, 512]
    dec_w = np.asarray(inputs["dec_w"], np.float32)
    dec_b = np.asarray(inputs["dec_b"], np.float32)
    stats = h_last.astype(np.float32) @ dec_w.T + dec_b
    m, v_ = np.split(stats, 2, axis=-1)
    v = _softplus(v_) + 1e-5
    return (m.astype(np.float32), v.astype(np.float32))
